# revision 9
# baseline (speedup 1.0000x reference)
"""RGCN-with-history (DGL RelGraphConv + history splice) on 8 TRN2 NeuronCores.

Key structural fact: the history splice dominates — out[n] is an exact copy of
history_buffer[history_map[n]] wherever history_map[n] >= 0, and the RGCN
aggregation only survives for the (very few) nodes with history_map[n] < 0.

Strategy (memory-bound regime):
  - Shard destination nodes across 8 cores (6250 each); each core
    indirect-gathers its 6250 history rows straight into the output staging
    tile (one dma_gather).
  - The globally-rare "no history" nodes are computed on every core
    (replicated tiny compute, keeps the SPMD program identical): their
    incoming edges are bucketed into 16-node chunks; per 128-edge tile we
    gather source features (bf16) and accumulate Z^T[64, 128] += Xg^T @ S
    where S is a (relation, node-rank) one-hot built on the vector engine.
    Relation weights + self-loop + bias are applied with small matmuls; the
    computed rows land in a DRAM scratch table.
  - A second per-core gather pulls each core's computed rows; a single
    predicated copy overlays them onto the history staging, which is DMA'd
    out as one block.
"""
import sys

sys.path.insert(0, "/opt/trn_rl_repo")

import numpy as np
import ml_dtypes

import concourse.bacc as bacc
import concourse.tile as tile
import concourse.mybir as mybir
from concourse.bass_utils import run_bass_kernel_spmd

BF16 = ml_dtypes.bfloat16

N_NODES = 50000
N_EDGES = 800000
CH = 64
N_REL = 8
BUF = 20000
N_CORES = 8
DPC = N_NODES // N_CORES            # 6250 dst nodes per core
NPAD = 6400                         # padded dst rows per core (50 x 128)
NCOL = NPAD // 128                  # 50 staging columns
SPLIT = 32767                       # src < SPLIT -> lo table, else hi
T0_ROWS = SPLIT + 1                 # lo table rows; row SPLIT is zeros
T1_ROWS = N_NODES - SPLIT + 1       # hi table rows; row 0 is zeros
CHUNK = 16                          # invalid nodes per compute chunk
BATCH = 4096                        # max gather indices per dma_gather

_cache = {}


def _wrap16(a):
    """Flat index array -> [128, len/16] int16 wrapped layout (idx k at
    [k%16, k//16], replicated across the 8 gpsimd lanes)."""
    m = a.reshape(-1, 16).T.astype(np.int16)
    return np.tile(m, (8, 1)).copy()


def _host_prep(x, W, loop_w, bias, history_buffer, src, dst, etypes, history_map):
    src = np.asarray(src)
    dst = np.asarray(dst)
    etypes = np.asarray(etypes)
    x = np.asarray(x, dtype=np.float32)
    hm = np.asarray(history_map)
    hb = np.asarray(history_buffer, np.float32)

    # --- shared gather tables (bf16, 128-col padded rows = 256B) ---
    tab0 = np.zeros((T0_ROWS, CH), np.float32)
    tab0[:SPLIT] = x[:SPLIT]
    tab1 = np.zeros((T1_ROWS, CH), np.float32)
    tab1[1:] = x[SPLIT:]

    # --- globally-rare invalid (no-history) nodes: replicated tiny compute ---
    inv_nodes = np.where(hm < 0)[0]              # sorted
    M = len(inv_nodes)
    NCHUNK = max(1, -(-M // CHUNK)) if M > 0 else 0
    MP = max(CHUNK, NCHUNK * CHUNK)              # scratch rows (>=16)

    n_lo = np.zeros(max(NCHUNK, 1), np.int64)
    n_hi = np.zeros(max(NCHUNK, 1), np.int64)
    idx_lo_slots = []
    idx_hi_slots = []
    srk_cols = None
    Tinv = 0
    chunk_tiles = []
    if M > 0:
        grank = np.full(N_NODES, -1, np.int64)
        grank[inv_nodes] = np.arange(M)
        emask = grank[dst] >= 0
        e_src = src[emask]
        e_et = etypes[emask]
        e_rank = grank[dst[emask]]
        e_chunk = e_rank // CHUNK
        e_half = (e_src >= SPLIT).astype(np.int64)
        e_col = e_et * CHUNK + (e_rank % CHUNK)  # one-hot col within chunk

        slots_lo = {}
        slots_hi = {}
        for ch in range(NCHUNK):
            for h, (slots, n_arr) in enumerate(((slots_lo, n_lo), (slots_hi, n_hi))):
                m = (e_chunk == ch) & (e_half == h)
                cnt = int(m.sum())
                n = -(-cnt // 128) if cnt else 0
                n_arr[ch] = n
                idxv = np.full(n * 128, SPLIT if h == 0 else 0, np.int64)
                srkv = np.zeros(n * 128, np.float32)
                s = e_src[m]
                idxv[:cnt] = s if h == 0 else s - (SPLIT - 1)
                srkv[:cnt] = e_col[m]
                slots[ch] = (idxv, srkv)
        # stream-concatenated slot arrays + srank columns in tile order
        srk_list = []
        for ch in range(NCHUNK):
            tl = []
            for h, slots in enumerate((slots_lo, slots_hi)):
                idxv, srkv = slots[ch]
                (idx_lo_slots if h == 0 else idx_hi_slots).append(idxv)
                for t in range(len(idxv) // 128):
                    srk_list.append(srkv[t * 128:(t + 1) * 128])
                    tl.append((h, t))
            chunk_tiles.append(tl)
        Tinv = len(srk_list)
        srk_cols = (np.stack(srk_list, axis=1) if Tinv
                    else np.zeros((128, 0), np.float32))

    SL0 = int(n_lo.sum()) * 128
    SL1 = int(n_hi.sum()) * 128
    SLP0 = max(128, SL0)
    SLP1 = max(128, SL1)
    idx_lo = np.full(SLP0, SPLIT, np.int64)
    idx_hi = np.zeros(SLP1, np.int64)
    if idx_lo_slots:
        cat = np.concatenate(idx_lo_slots) if SL0 else idx_lo[:0]
        idx_lo[:SL0] = cat
    if idx_hi_slots:
        cat = np.concatenate(idx_hi_slots) if SL1 else idx_hi[:0]
        idx_hi[:SL1] = cat
    TinvP = max(1, Tinv)
    srk = np.zeros((128, TinvP), np.float32)
    if Tinv:
        srk[:, :Tinv] = srk_cols

    meta = {
        "M": M, "NCHUNK": NCHUNK, "MP": MP, "Tinv": Tinv, "TinvP": TinvP,
        "n_lo": n_lo, "n_hi": n_hi, "chunk_tiles": chunk_tiles,
        "SLP0": SLP0, "SLP1": SLP1,
    }

    # --- weights / constants (shared) ---
    Wsb = np.zeros((64, N_REL, CH), np.float32)
    for r in range(N_REL):
        Wsb[:, r, :] = np.asarray(W[r], np.float32)
    lwa = np.zeros((128, CH), np.float32)
    lwa[:CH] = np.asarray(loop_w, np.float32)
    lwa[CH] = np.asarray(bias, np.float32)
    iota = np.tile(np.arange(128, dtype=np.float32)[None, :], (128, 1)).copy()
    xti = np.zeros((128, MP), np.float32)
    if M:
        xti[:CH, :M] = x[inv_nodes].T
        xti[CH, :M] = 1.0

    shared = {
        "tab0": tab0, "tab1": tab1, "idx_lo": _wrap16(idx_lo),
        "idx_hi": _wrap16(idx_hi), "srk": srk, "iota": iota,
        "wsb": Wsb, "lwa": lwa, "xti": xti, "hbuf": hb,
    }

    in_maps = []
    for c in range(N_CORES):
        hm_loc = np.zeros(NPAD, np.int64)
        hm_loc[:DPC] = hm[c * DPC:(c + 1) * DPC]
        hidx = np.clip(hm_loc, 0, BUF - 1)
        valid = hm_loc >= 0
        valid[DPC:] = True               # pad rows: treat as "history" side
        # one-hot selector routing computed row grank -> local position n
        sel = np.zeros((CHUNK, max(NCHUNK, 1) * NPAD), np.float32)
        if M:
            gr = grank[c * DPC:(c + 1) * DPC]
            loc_inv = np.where(gr >= 0)[0]
            for n in loc_inv:
                r = int(gr[n])
                sel[r % CHUNK, (r // CHUNK) * NPAD + n] = 1.0
        invmask = np.zeros((128, NCOL, CH), np.uint8)
        invmask[:, :, :] = (~valid).reshape(-1, 128).T[:, :, None]
        in_maps.append({
            **shared,
            "hidx": _wrap16(hidx), "sel": sel, "invmask": invmask,
        })
    return meta, in_maps


def _build_program(meta):
    M, NCHUNK, MP = meta["M"], meta["NCHUNK"], meta["MP"]
    TinvP = meta["TinvP"]
    n_lo, n_hi = meta["n_lo"], meta["n_hi"]
    SLP0, SLP1 = meta["SLP0"], meta["SLP1"]

    nc = bacc.Bacc("TRN2", target_bir_lowering=False, debug=False,
                   num_devices=N_CORES)
    dt = mybir.dt
    d_tab0 = nc.dram_tensor("tab0", [T0_ROWS, CH], dt.float32, kind="ExternalInput")
    d_tab1 = nc.dram_tensor("tab1", [T1_ROWS, CH], dt.float32, kind="ExternalInput")
    d_ilo = nc.dram_tensor("idx_lo", [128, SLP0 // 16], dt.int16, kind="ExternalInput")
    d_ihi = nc.dram_tensor("idx_hi", [128, SLP1 // 16], dt.int16, kind="ExternalInput")
    d_srk = nc.dram_tensor("srk", [128, TinvP], dt.float32, kind="ExternalInput")
    d_iota = nc.dram_tensor("iota", [128, 128], dt.float32, kind="ExternalInput")
    d_wsb = nc.dram_tensor("wsb", [64, N_REL, CH], dt.float32, kind="ExternalInput")
    d_lwa = nc.dram_tensor("lwa", [128, CH], dt.float32, kind="ExternalInput")
    d_xti = nc.dram_tensor("xti", [128, MP], dt.float32, kind="ExternalInput")
    d_hbuf = nc.dram_tensor("hbuf", [BUF, CH], dt.float32, kind="ExternalInput")
    d_hidx = nc.dram_tensor("hidx", [128, NPAD // 16], dt.int16, kind="ExternalInput")
    d_sel = nc.dram_tensor("sel", [CHUNK, max(NCHUNK, 1) * NPAD], dt.float32,
                           kind="ExternalInput")
    d_invm = nc.dram_tensor("invmask", [128, NCOL, CH], dt.uint8,
                            kind="ExternalInput")
    d_out = nc.dram_tensor("out", [NPAD, CH], dt.float32, kind="ExternalOutput")

    with tile.TileContext(nc) as tc:
        with (
            tc.tile_pool(name="const", bufs=1) as cpool,
            tc.tile_pool(name="g", bufs=2) as gpool,
            tc.tile_pool(name="s", bufs=2) as spool,
            tc.tile_pool(name="pz", bufs=2, space="PSUM") as pzpool,
            tc.tile_pool(name="po", bufs=2, space="PSUM") as popool,
            tc.tile_pool(name="pov", bufs=2, space="PSUM") as povpool,
        ):
            hidx_sb = cpool.tile([128, NPAD // 16], dt.int16)
            nc.sync.dma_start(hidx_sb[:], d_hidx[:])
            stage = cpool.tile([128, NCOL, CH], dt.float32)
            nc.gpsimd.dma_gather(
                stage[:], d_hbuf[:], hidx_sb[:],
                num_idxs=NPAD, num_idxs_reg=NPAD, elem_size=CH,
                single_packet=False,
            )

            if M > 0:
                ilo_sb = cpool.tile([128, SLP0 // 16], dt.int16)
                ihi_sb = cpool.tile([128, SLP1 // 16], dt.int16)
                srk_sb = cpool.tile([128, TinvP], dt.float32)
                iota_sb = cpool.tile([128, 128], dt.float32)
                wsb_sb = cpool.tile([64, N_REL, CH], dt.float32)
                lwa_sb = cpool.tile([128, CH], dt.float32)
                xti_sb = cpool.tile([128, MP], dt.float32)
                sel_sb = cpool.tile([CHUNK, max(NCHUNK, 1) * NPAD], dt.float32)
                invm_sb = cpool.tile([128, NCOL, CH], dt.uint8)
                for t_sb, t_d in ((ilo_sb, d_ilo), (ihi_sb, d_ihi),
                                  (srk_sb, d_srk), (iota_sb, d_iota),
                                  (wsb_sb, d_wsb), (lwa_sb, d_lwa),
                                  (xti_sb, d_xti), (sel_sb, d_sel),
                                  (invm_sb, d_invm)):
                    nc.sync.dma_start(t_sb[:], t_d[:])

                # per-(chunk, half) source-feature gathers
                gtiles = {}
                off = {0: 0, 1: 0}
                for ch in range(NCHUNK):
                    for h, n in ((0, int(n_lo[ch])), (1, int(n_hi[ch]))):
                        if n == 0:
                            continue
                        assert n * 128 <= BATCH
                        g = gpool.tile([128, n, CH], dt.float32, tag="g",
                                       name=f"g_{ch}_{h}")
                        tab = d_tab0 if h == 0 else d_tab1
                        isb = ilo_sb if h == 0 else ihi_sb
                        o = off[h]
                        nc.gpsimd.dma_gather(
                            g[:], tab[:], isb[:, o // 16:(o + n * 128) // 16],
                            num_idxs=n * 128, num_idxs_reg=n * 128,
                            elem_size=CH, single_packet=False,
                        )
                        gtiles[(ch, h)] = g
                        off[h] += n * 128

                gt = 0
                cps = []
                for ch in range(NCHUNK):
                    tl = meta["chunk_tiles"][ch]
                    ntot = len(tl)
                    if ntot:
                        pz = pzpool.tile([64, 128], dt.float32, tag="pz",
                                         name=f"pz_{ch}")
                        for i, (h, t) in enumerate(tl):
                            S = spool.tile([128, 128], dt.float32, tag="S",
                                           name=f"S_{ch}_{i}")
                            nc.vector.tensor_scalar(
                                S[:], iota_sb[:], srk_sb[:, gt:gt + 1], None,
                                mybir.AluOpType.is_equal,
                            )
                            xg = gtiles[(ch, h)][:, t, 0:CH]
                            nc.tensor.matmul(pz[:], xg, S[:],
                                             start=(i == 0),
                                             stop=(i == ntot - 1))
                            gt += 1
                        zt = spool.tile([64, 128], dt.float32, tag="zt",
                                        name=f"zt_{ch}")
                        nc.scalar.activation(zt[:], pz[:],
                                             mybir.ActivationFunctionType.Copy)
                    po = popool.tile([CHUNK, CH], dt.float32, tag="po",
                                     name=f"po_{ch}")
                    nc.tensor.matmul(po[:], xti_sb[:, ch * CHUNK:(ch + 1) * CHUNK],
                                     lwa_sb[:], start=True, stop=(ntot == 0))
                    if ntot:
                        for r in range(N_REL):
                            nc.tensor.matmul(
                                po[:], zt[:, r * CHUNK:(r + 1) * CHUNK],
                                wsb_sb[:, r, :], start=False,
                                stop=(r == N_REL - 1),
                            )
                    cp = cpool.tile([CHUNK, CH], dt.float32,
                                    name=f"cp_{ch}")
                    nc.vector.tensor_copy(cp[:], po[:])
                    cps.append(cp)

                # route computed rows to their local positions via one-hot
                # selector matmuls, then overlay onto the history staging
                for cb in range(NCOL):
                    pov = povpool.tile([128, CH], dt.float32, tag="pov",
                                       name=f"pov_{cb}")
                    for ch in range(NCHUNK):
                        nc.tensor.matmul(
                            pov[:],
                            sel_sb[:, ch * NPAD + cb * 128:
                                   ch * NPAD + cb * 128 + 128],
                            cps[ch][:], start=(ch == 0),
                            stop=(ch == NCHUNK - 1),
                        )
                    nc.vector.copy_predicated(stage[:, cb, :],
                                              invm_sb[:, cb, :], pov[:])

            nc.sync.dma_start(
                d_out.ap().rearrange("(j p) c -> p j c", p=128), stage[:]
            )
    nc.compile()
    return nc


def _run(inputs, trace=False):
    meta, in_maps = _host_prep(**inputs)
    key = ("prog", meta["M"], meta["NCHUNK"], meta["Tinv"])
    if key not in _cache:
        _cache[key] = _build_program(meta)
    nc = _cache[key]
    res = run_bass_kernel_spmd(nc, in_maps, list(range(N_CORES)), trace=trace)
    out = np.concatenate(
        [res.results[c]["out"][:DPC] for c in range(N_CORES)], axis=0
    ).astype(np.float32)
    return out, res


def kernel(**inputs):
    out, _ = _run(inputs)
    return out


# revision 22
# speedup vs baseline: 1.7879x; 1.7879x over previous
"""RGCN-with-history (DGL RelGraphConv + history splice) on 8 TRN2 NeuronCores.

Key structural fact: the history splice dominates — out[n] is an exact copy of
history_buffer[history_map[n]] wherever history_map[n] >= 0, and the RGCN
aggregation only survives for the (very few) nodes with history_map[n] < 0.

Strategy (memory-bound regime):
  - Shard destination nodes across 8 cores (6250 each); each core
    indirect-gathers its history rows straight into two output staging
    halves (two dma_gathers, pipelined with the two output DMAs).
  - The globally-rare "no history" nodes are computed on every core
    (replicated tiny fp32 compute keeps the SPMD program identical): their
    incoming edges are bucketed into 16-node chunks; per 128-edge tile we
    indirect-gather source features and accumulate Z^T[64, 128] += Xg^T @ S
    on the tensor engine, where S is a (relation, node-rank) one-hot built
    on the vector engine (is_equal against an iota row). Relation weights +
    self-loop + bias are applied with small matmuls.
  - Computed rows are routed to their data-dependent positions with one-hot
    selector matmuls (only for the few staging columns that contain such a
    node on any core) and overlaid onto the history staging via predicated
    copies. Everything stays on-chip; no DRAM round-trip.
"""
import sys

sys.path.insert(0, "/opt/trn_rl_repo")

import numpy as np

import concourse.bacc as bacc
import concourse.tile as tile
import concourse.mybir as mybir
from concourse.bass_utils import run_bass_kernel_spmd

N_NODES = 50000
N_EDGES = 800000
CH = 64
N_REL = 8
BUF = 20000
N_CORES = 8
DPC = N_NODES // N_CORES            # 6250 dst nodes per core
NPAD = 6400                         # padded dst rows per core (50 x 128)
NCOL = NPAD // 128                  # 50 staging columns
SPLIT = 32767                       # src < SPLIT -> lo table, else hi
T0_ROWS = SPLIT + 1                 # lo table rows; row SPLIT is zeros
T1_ROWS = N_NODES - SPLIT + 1       # hi table rows; row 0 is zeros
CHUNK = 16                          # invalid nodes per compute chunk
BATCH = 4096                        # max gather indices per dma_gather

_cache = {}


def _wrap16(a):
    """Flat index array -> [128, len/16] int16 wrapped layout (idx k at
    [k%16, k//16], replicated across the 8 gpsimd lanes)."""
    m = a.reshape(-1, 16).T.astype(np.int16)
    return np.tile(m, (8, 1)).copy()


def _host_prep(x, W, loop_w, bias, history_buffer, src, dst, etypes, history_map):
    src = np.asarray(src)
    dst = np.asarray(dst)
    etypes = np.asarray(etypes)
    x = np.asarray(x, dtype=np.float32)
    hm = np.asarray(history_map)
    hb = np.asarray(history_buffer, np.float32)

    # --- shared gather tables (bf16, 128-col padded rows = 256B) ---
    tab0 = np.zeros((T0_ROWS, CH), np.float32)
    tab0[:SPLIT] = x[:SPLIT]
    tab1 = np.zeros((T1_ROWS, CH), np.float32)
    tab1[1:] = x[SPLIT:]

    # --- globally-rare invalid (no-history) nodes: replicated tiny compute ---
    inv_nodes = np.where(hm < 0)[0]              # sorted
    M = len(inv_nodes)
    NCHUNK = max(1, -(-M // CHUNK)) if M > 0 else 0
    MP = max(CHUNK, NCHUNK * CHUNK)              # scratch rows (>=16)

    n_lo = np.zeros(max(NCHUNK, 1), np.int64)
    n_hi = np.zeros(max(NCHUNK, 1), np.int64)
    idx_lo_slots = []
    idx_hi_slots = []
    srk_cols = None
    Tinv = 0
    chunk_tiles = []
    if M > 0:
        grank = np.full(N_NODES, -1, np.int64)
        grank[inv_nodes] = np.arange(M)
        emask = grank[dst] >= 0
        e_src = src[emask]
        e_et = etypes[emask]
        e_rank = grank[dst[emask]]
        e_chunk = e_rank // CHUNK
        e_half = (e_src >= SPLIT).astype(np.int64)
        e_col = e_et * CHUNK + (e_rank % CHUNK)  # one-hot col within chunk

        slots_lo = {}
        slots_hi = {}
        for ch in range(NCHUNK):
            for h, (slots, n_arr) in enumerate(((slots_lo, n_lo), (slots_hi, n_hi))):
                m = (e_chunk == ch) & (e_half == h)
                cnt = int(m.sum())
                n = -(-cnt // 128) if cnt else 0
                n_arr[ch] = n
                idxv = np.full(n * 128, SPLIT if h == 0 else 0, np.int64)
                srkv = np.zeros(n * 128, np.float32)
                s = e_src[m]
                idxv[:cnt] = s if h == 0 else s - (SPLIT - 1)
                srkv[:cnt] = e_col[m]
                slots[ch] = (idxv, srkv)
        # stream-concatenated slot arrays + srank columns in tile order
        srk_list = []
        for ch in range(NCHUNK):
            tl = []
            for h, slots in enumerate((slots_lo, slots_hi)):
                idxv, srkv = slots[ch]
                (idx_lo_slots if h == 0 else idx_hi_slots).append(idxv)
                for t in range(len(idxv) // 128):
                    srk_list.append(srkv[t * 128:(t + 1) * 128])
                    tl.append((h, t))
            chunk_tiles.append(tl)
        Tinv = len(srk_list)
        srk_cols = (np.stack(srk_list, axis=1) if Tinv
                    else np.zeros((128, 0), np.float32))

    SL0 = int(n_lo.sum()) * 128
    SL1 = int(n_hi.sum()) * 128
    SLP0 = max(128, SL0)
    SLP1 = max(128, SL1)
    idx_lo = np.full(SLP0, SPLIT, np.int64)
    idx_hi = np.zeros(SLP1, np.int64)
    if idx_lo_slots:
        cat = np.concatenate(idx_lo_slots) if SL0 else idx_lo[:0]
        idx_lo[:SL0] = cat
    if idx_hi_slots:
        cat = np.concatenate(idx_hi_slots) if SL1 else idx_hi[:0]
        idx_hi[:SL1] = cat
    TinvP = max(1, Tinv)
    srk = np.zeros((128, TinvP), np.float32)
    if Tinv:
        srk[:, :Tinv] = srk_cols

    # union (over cores) of staging columns that hold an invalid node —
    # only these columns need the computed-row overlay
    if M:
        inv_local = inv_nodes % DPC
        cols_used = sorted(set((inv_local // 128).tolist()))
    else:
        cols_used = []

    meta = {
        "M": M, "NCHUNK": NCHUNK, "MP": MP, "Tinv": Tinv, "TinvP": TinvP,
        "n_lo": n_lo, "n_hi": n_hi, "chunk_tiles": chunk_tiles,
        "SLP0": SLP0, "SLP1": SLP1, "cols_used": tuple(cols_used),
    }

    # --- weights / constants (shared) ---
    Wsb = np.zeros((64, N_REL, CH), np.float32)
    for r in range(N_REL):
        Wsb[:, r, :] = np.asarray(W[r], np.float32)
    lwa = np.zeros((128, CH), np.float32)
    lwa[:CH] = np.asarray(loop_w, np.float32)
    lwa[CH] = np.asarray(bias, np.float32)
    iota = np.tile(np.arange(128, dtype=np.float32)[None, :], (128, 1)).copy()
    xti = np.zeros((128, MP), np.float32)
    if M:
        xti[:CH, :M] = x[inv_nodes].T
        xti[CH, :M] = 1.0

    # merge the small f32 constants into one array (fewer DMAs):
    # [srk | iota(128) | lwa(64) | xti(MP) | wsb(512, rows 0:64)]
    cmega = np.zeros((128, TinvP + 128 + CH + MP + N_REL * CH), np.float32)
    o = 0
    cmega[:, o:o + TinvP] = srk; o += TinvP
    cmega[:, o:o + 128] = iota; o += 128
    cmega[:, o:o + CH] = lwa; o += CH
    cmega[:, o:o + MP] = xti; o += MP
    cmega[:64, o:o + N_REL * CH] = Wsb.reshape(64, N_REL * CH)

    shared = {
        "tab0": tab0, "tab1": tab1, "idx_lo": _wrap16(idx_lo),
        "idx_hi": _wrap16(idx_hi), "cmega": cmega, "hbuf": hb,
    }

    in_maps = []
    for c in range(N_CORES):
        hm_loc = np.zeros(NPAD, np.int64)
        hm_loc[:DPC] = hm[c * DPC:(c + 1) * DPC]
        hidx = np.clip(hm_loc, 0, BUF - 1)
        valid = hm_loc >= 0
        valid[DPC:] = True               # pad rows: treat as "history" side
        # one-hot selector routing computed row grank -> local position n
        sel = np.zeros((CHUNK, max(NCHUNK, 1) * NPAD), np.float32)
        if M:
            gr = grank[c * DPC:(c + 1) * DPC]
            loc_inv = np.where(gr >= 0)[0]
            for n in loc_inv:
                rr = int(gr[n])
                sel[rr % CHUNK, (rr // CHUNK) * NPAD + n] = 1.0
        invmask = np.zeros((128, NCOL, CH), np.uint8)
        invmask[:, :, :] = (~valid).reshape(-1, 128).T[:, :, None]
        in_maps.append({
            **shared,
            "hidx": _wrap16(hidx), "sel": sel, "invmask": invmask,
        })
    return meta, in_maps


def _build_program(meta):
    M, NCHUNK, MP = meta["M"], meta["NCHUNK"], meta["MP"]
    TinvP = meta["TinvP"]
    n_lo, n_hi = meta["n_lo"], meta["n_hi"]
    SLP0, SLP1 = meta["SLP0"], meta["SLP1"]
    CMW = TinvP + 128 + CH + MP + N_REL * CH
    HALF = NCOL // 2                     # staging split for pipelining

    nc = bacc.Bacc("TRN2", target_bir_lowering=False, debug=False,
                   num_devices=N_CORES)
    dt = mybir.dt
    d_tab0 = nc.dram_tensor("tab0", [T0_ROWS, CH], dt.float32, kind="ExternalInput")
    d_tab1 = nc.dram_tensor("tab1", [T1_ROWS, CH], dt.float32, kind="ExternalInput")
    d_ilo = nc.dram_tensor("idx_lo", [128, SLP0 // 16], dt.int16, kind="ExternalInput")
    d_ihi = nc.dram_tensor("idx_hi", [128, SLP1 // 16], dt.int16, kind="ExternalInput")
    d_cm = nc.dram_tensor("cmega", [128, CMW], dt.float32, kind="ExternalInput")
    d_hbuf = nc.dram_tensor("hbuf", [BUF, CH], dt.float32, kind="ExternalInput")
    d_hidx = nc.dram_tensor("hidx", [128, NPAD // 16], dt.int16, kind="ExternalInput")
    d_sel = nc.dram_tensor("sel", [CHUNK, max(NCHUNK, 1) * NPAD], dt.float32,
                           kind="ExternalInput")
    d_invm = nc.dram_tensor("invmask", [128, NCOL, CH], dt.uint8,
                            kind="ExternalInput")
    d_out = nc.dram_tensor("out", [128, NCOL, CH], dt.float32, kind="ExternalOutput")

    with tile.TileContext(nc) as tc:
        with (
            tc.tile_pool(name="const", bufs=1) as cpool,
            tc.tile_pool(name="g", bufs=2) as gpool,
            tc.tile_pool(name="s", bufs=2) as spool,
            tc.tile_pool(name="pz", bufs=2, space="PSUM") as pzpool,
            tc.tile_pool(name="po", bufs=2, space="PSUM") as popool,
            tc.tile_pool(name="pov", bufs=2, space="PSUM") as povpool,
        ):
            hidx_sb = cpool.tile([128, NPAD // 16], dt.int16)
            nc.sync.dma_start(hidx_sb[:], d_hidx[:])
            # two staging halves -> history gather and output DMA pipeline
            stages = [cpool.tile([128, HALF, CH], dt.float32, name="stageA"),
                      cpool.tile([128, NCOL - HALF, CH], dt.float32,
                                 name="stageB")]
            nidx = (HALF * 128, (NCOL - HALF) * 128)
            for half in range(2):
                o = half * HALF * 8      # idx cols consumed (128 idx / 8 col)
                nc.gpsimd.dma_gather(
                    stages[half][:], d_hbuf[:],
                    hidx_sb[:, o:o + nidx[half] // 16],
                    num_idxs=nidx[half], num_idxs_reg=nidx[half],
                    elem_size=CH, single_packet=False,
                )

            if M > 0:
                ilo_sb = cpool.tile([128, SLP0 // 16], dt.int16)
                ihi_sb = cpool.tile([128, SLP1 // 16], dt.int16)
                cm_sb = cpool.tile([128, CMW], dt.float32)
                sel_sb = cpool.tile([CHUNK, max(NCHUNK, 1) * NPAD], dt.float32)
                invm_sb = cpool.tile([128, NCOL, CH], dt.uint8)
                for i, (t_sb, t_d) in enumerate(
                        ((cm_sb, d_cm), (sel_sb, d_sel), (ilo_sb, d_ilo),
                         (ihi_sb, d_ihi), (invm_sb, d_invm))):
                    eng = nc.sync if i % 2 == 0 else nc.scalar
                    eng.dma_start(t_sb[:], t_d[:])
                o = 0
                srk_sb = cm_sb[:, 0:TinvP]; o = TinvP
                iota_sb = cm_sb[:, o:o + 128]; o += 128
                lwa_sb = cm_sb[:, o:o + CH]; o += CH
                xti_sb = cm_sb[:, o:o + MP]; o += MP
                wsb_o = o

                # per-(chunk, half) source-feature gathers
                gtiles = {}
                off = {0: 0, 1: 0}
                for ch in range(NCHUNK):
                    for h, n in ((0, int(n_lo[ch])), (1, int(n_hi[ch]))):
                        if n == 0:
                            continue
                        assert n * 128 <= BATCH
                        g = gpool.tile([128, n, CH], dt.float32, tag="g",
                                       name=f"g_{ch}_{h}")
                        tab = d_tab0 if h == 0 else d_tab1
                        isb = ilo_sb if h == 0 else ihi_sb
                        o2 = off[h]
                        nc.gpsimd.dma_gather(
                            g[:], tab[:], isb[:, o2 // 16:(o2 + n * 128) // 16],
                            num_idxs=n * 128, num_idxs_reg=n * 128,
                            elem_size=CH, single_packet=False,
                        )
                        gtiles[(ch, h)] = g
                        off[h] += n * 128

                gt = 0
                cps = []
                for ch in range(NCHUNK):
                    tl = meta["chunk_tiles"][ch]
                    ntot = len(tl)
                    if ntot:
                        pz = pzpool.tile([64, 128], dt.float32, tag="pz",
                                         name=f"pz_{ch}")
                        for i, (h, t) in enumerate(tl):
                            S = spool.tile([128, 128], dt.float32, tag="S",
                                           name=f"S_{ch}_{i}")
                            nc.vector.tensor_scalar(
                                S[:], iota_sb, srk_sb[:, gt:gt + 1], None,
                                mybir.AluOpType.is_equal,
                            )
                            xg = gtiles[(ch, h)][:, t, 0:CH]
                            nc.tensor.matmul(pz[:], xg, S[:],
                                             start=(i == 0),
                                             stop=(i == ntot - 1))
                            gt += 1
                        zt = spool.tile([64, 128], dt.float32, tag="zt",
                                        name=f"zt_{ch}")
                        nc.scalar.activation(zt[:], pz[:],
                                             mybir.ActivationFunctionType.Copy)
                    po = popool.tile([CHUNK, CH], dt.float32, tag="po",
                                     name=f"po_{ch}")
                    nc.tensor.matmul(po[:], xti_sb[:, ch * CHUNK:(ch + 1) * CHUNK],
                                     lwa_sb, start=True, stop=(ntot == 0))
                    if ntot:
                        for r in range(N_REL):
                            nc.tensor.matmul(
                                po[:], zt[:, r * CHUNK:(r + 1) * CHUNK],
                                cm_sb[0:64, wsb_o + r * CH:wsb_o + (r + 1) * CH],
                                start=False, stop=(r == N_REL - 1),
                            )
                    cp = cpool.tile([CHUNK, CH], dt.float32,
                                    name=f"cp_{ch}")
                    nc.vector.tensor_copy(cp[:], po[:])
                    cps.append(cp)

                # route computed rows to their positions; only columns that
                # hold an invalid node on some core need the overlay
                for cb in meta["cols_used"]:
                    pov = povpool.tile([128, CH], dt.float32, tag="pov",
                                       name=f"pov_{cb}")
                    for ch in range(NCHUNK):
                        nc.tensor.matmul(
                            pov[:],
                            sel_sb[:, ch * NPAD + cb * 128:
                                   ch * NPAD + cb * 128 + 128],
                            cps[ch][:], start=(ch == 0),
                            stop=(ch == NCHUNK - 1),
                        )
                    half, lc = (0, cb) if cb < HALF else (1, cb - HALF)
                    nc.vector.copy_predicated(stages[half][:, lc, :],
                                              invm_sb[:, cb, :], pov[:])

            nc.sync.dma_start(d_out[:, 0:HALF, :], stages[0][:])
            nc.scalar.dma_start(d_out[:, HALF:NCOL, :], stages[1][:])
    nc.compile()
    return nc


def _prog_key(meta):
    return ("prog", meta["M"], meta["NCHUNK"], meta["Tinv"],
            tuple(meta["n_lo"]), tuple(meta["n_hi"]), meta["cols_used"])


def _run(inputs, trace=False):
    meta, in_maps = _host_prep(**inputs)
    key = _prog_key(meta)
    if key not in _cache:
        _cache[key] = _build_program(meta)
    nc = _cache[key]
    res = run_bass_kernel_spmd(nc, in_maps, list(range(N_CORES)), trace=trace)
    out = np.concatenate(
        [res.results[c]["out"].transpose(1, 0, 2).reshape(NPAD, CH)[:DPC]
         for c in range(N_CORES)], axis=0
    ).astype(np.float32)
    return out, res


def kernel(**inputs):
    out, _ = _run(inputs)
    return out


# revision 30
# speedup vs baseline: 2.1147x; 1.1828x over previous
"""RGCN-with-history (DGL RelGraphConv + history splice) on 8 TRN2 NeuronCores.

Key structural fact: the history splice dominates — out[n] is an exact copy of
history_buffer[history_map[n]] wherever history_map[n] >= 0, and the RGCN
aggregation only survives for the (very few) nodes with history_map[n] < 0.

Strategy (memory-bound regime):
  - Shard destination nodes across 8 cores (6250 each); each core
    indirect-gathers its history rows straight into two output staging
    halves (two dma_gathers, pipelined with the two output DMAs).
  - The globally-rare "no history" nodes are computed on every core
    (replicated tiny fp32 compute keeps the SPMD program identical): their
    incoming edges are bucketed into 16-node chunks; per 128-edge tile we
    indirect-gather source features and accumulate Z^T[64, 128] += Xg^T @ S
    on the tensor engine, where S is a (relation, node-rank) one-hot built
    on the vector engine (is_equal against an iota row). Relation weights +
    self-loop + bias are applied with small matmuls.
  - Computed rows are routed to their data-dependent positions with one-hot
    selector matmuls (only for the few staging columns that contain such a
    node on any core) and overlaid onto the history staging via predicated
    copies. Everything stays on-chip; no DRAM round-trip.
"""
import sys

sys.path.insert(0, "/opt/trn_rl_repo")

import numpy as np

import concourse.bacc as bacc
import concourse.tile as tile
import concourse.mybir as mybir
from concourse.bass_utils import run_bass_kernel_spmd

N_NODES = 50000
N_EDGES = 800000
CH = 64
N_REL = 8
BUF = 20000
N_CORES = 8
DPC = N_NODES // N_CORES            # 6250 dst nodes per core
NPAD = 6400                         # padded dst rows per core (50 x 128)
NCOL = NPAD // 128                  # 50 staging columns
SPLIT = 32767                       # src < SPLIT -> lo table, else hi
T0_ROWS = SPLIT + 1                 # lo table rows; row SPLIT is zeros
T1_ROWS = N_NODES - SPLIT + 1       # hi table rows; row 0 is zeros
CHUNK = 16                          # invalid nodes per compute chunk
BATCH = 4096                        # max gather indices per dma_gather

_cache = {}


def _wrap16(a):
    """Flat index array -> [128, len/16] int16 wrapped layout (idx k at
    [k%16, k//16], replicated across the 8 gpsimd lanes)."""
    m = a.reshape(-1, 16).T.astype(np.int16)
    return np.tile(m, (8, 1)).copy()


def _host_prep(x, W, loop_w, bias, history_buffer, src, dst, etypes, history_map):
    src = np.asarray(src)
    dst = np.asarray(dst)
    etypes = np.asarray(etypes)
    x = np.asarray(x, dtype=np.float32)
    hm = np.asarray(history_map)
    hb = np.asarray(history_buffer, np.float32)

    # --- shared gather tables (bf16, 128-col padded rows = 256B) ---
    tab0 = np.zeros((T0_ROWS, CH), np.float32)
    tab0[:SPLIT] = x[:SPLIT]
    tab1 = np.zeros((T1_ROWS, CH), np.float32)
    tab1[1:] = x[SPLIT:]

    # --- globally-rare invalid (no-history) nodes: replicated tiny compute ---
    inv_nodes = np.where(hm < 0)[0]              # sorted
    M = len(inv_nodes)
    NCHUNK = max(1, -(-M // CHUNK)) if M > 0 else 0
    MP = max(CHUNK, NCHUNK * CHUNK)              # scratch rows (>=16)

    n_lo = np.zeros(max(NCHUNK, 1), np.int64)
    n_hi = np.zeros(max(NCHUNK, 1), np.int64)
    idx_lo_slots = []
    idx_hi_slots = []
    srk_cols = None
    Tinv = 0
    chunk_tiles = []
    if M > 0:
        grank = np.full(N_NODES, -1, np.int64)
        grank[inv_nodes] = np.arange(M)
        emask = grank[dst] >= 0
        e_src = src[emask]
        e_et = etypes[emask]
        e_rank = grank[dst[emask]]
        e_chunk = e_rank // CHUNK
        e_half = (e_src >= SPLIT).astype(np.int64)
        e_col = e_et * CHUNK + (e_rank % CHUNK)  # one-hot col within chunk

        slots_lo = {}
        slots_hi = {}
        for ch in range(NCHUNK):
            for h, (slots, n_arr) in enumerate(((slots_lo, n_lo), (slots_hi, n_hi))):
                m = (e_chunk == ch) & (e_half == h)
                cnt = int(m.sum())
                n = -(-cnt // 128) if cnt else 0
                n_arr[ch] = n
                idxv = np.full(n * 128, SPLIT if h == 0 else 0, np.int64)
                srkv = np.zeros(n * 128, np.float32)
                s = e_src[m]
                idxv[:cnt] = s if h == 0 else s - (SPLIT - 1)
                srkv[:cnt] = e_col[m]
                slots[ch] = (idxv, srkv)
        # stream-concatenated slot arrays + srank columns in tile order
        srk_list = []
        for ch in range(NCHUNK):
            tl = []
            for h, slots in enumerate((slots_lo, slots_hi)):
                idxv, srkv = slots[ch]
                (idx_lo_slots if h == 0 else idx_hi_slots).append(idxv)
                for t in range(len(idxv) // 128):
                    srk_list.append(srkv[t * 128:(t + 1) * 128])
                    tl.append((h, t))
            chunk_tiles.append(tl)
        Tinv = len(srk_list)
        srk_cols = (np.stack(srk_list, axis=1) if Tinv
                    else np.zeros((128, 0), np.float32))

    SL0 = int(n_lo.sum()) * 128
    SL1 = int(n_hi.sum()) * 128
    SLP0 = max(128, SL0)
    SLP1 = max(128, SL1)
    idx_lo = np.full(SLP0, SPLIT, np.int64)
    idx_hi = np.zeros(SLP1, np.int64)
    if idx_lo_slots:
        cat = np.concatenate(idx_lo_slots) if SL0 else idx_lo[:0]
        idx_lo[:SL0] = cat
    if idx_hi_slots:
        cat = np.concatenate(idx_hi_slots) if SL1 else idx_hi[:0]
        idx_hi[:SL1] = cat
    TinvP = max(1, Tinv)
    srk = np.zeros((128, TinvP), np.float32)
    if Tinv:
        srk[:, :Tinv] = srk_cols

    # union (over cores) of staging columns that hold an invalid node —
    # only these columns need the computed-row overlay
    if M:
        inv_local = inv_nodes % DPC
        cols_used = sorted(set((inv_local // 128).tolist()))
    else:
        cols_used = []

    meta = {
        "M": M, "NCHUNK": NCHUNK, "MP": MP, "Tinv": Tinv, "TinvP": TinvP,
        "n_lo": n_lo, "n_hi": n_hi, "chunk_tiles": chunk_tiles,
        "SLP0": SLP0, "SLP1": SLP1, "cols_used": tuple(cols_used),
    }

    # --- weights / constants (shared) ---
    Wsb = np.zeros((64, N_REL, CH), np.float32)
    for r in range(N_REL):
        Wsb[:, r, :] = np.asarray(W[r], np.float32)
    lwa = np.zeros((128, CH), np.float32)
    lwa[:CH] = np.asarray(loop_w, np.float32)
    lwa[CH] = np.asarray(bias, np.float32)
    iota = np.tile(np.arange(128, dtype=np.float32)[None, :], (128, 1)).copy()
    xti = np.zeros((128, MP), np.float32)
    if M:
        xti[:CH, :M] = x[inv_nodes].T
        xti[CH, :M] = 1.0

    # merge the small f32 constants into one array (fewer DMAs):
    # [srk | iota(128) | lwa(64) | xti(MP) | wsb(512, rows 0:64)]
    cmega = np.zeros((128, TinvP + 128 + CH + MP + N_REL * CH), np.float32)
    o = 0
    cmega[:, o:o + TinvP] = srk; o += TinvP
    cmega[:, o:o + 128] = iota; o += 128
    cmega[:, o:o + CH] = lwa; o += CH
    cmega[:, o:o + MP] = xti; o += MP
    cmega[:64, o:o + N_REL * CH] = Wsb.reshape(64, N_REL * CH)

    shared = {
        "tab0": tab0, "tab1": tab1, "idx_lo": _wrap16(idx_lo),
        "idx_hi": _wrap16(idx_hi), "cmega": cmega, "hbuf": hb,
    }

    in_maps = []
    for c in range(N_CORES):
        hm_loc = np.zeros(NPAD, np.int64)
        hm_loc[:DPC] = hm[c * DPC:(c + 1) * DPC]
        hidx = np.clip(hm_loc, 0, BUF - 1)
        valid = hm_loc >= 0
        valid[DPC:] = True               # pad rows: treat as "history" side
        # selector + mask shipped only for the staging columns in cols_used
        NCU = max(len(cols_used), 1)
        sel = np.zeros((CHUNK, max(NCHUNK, 1) * NCU * 128), np.float32)
        invmask = np.zeros((128, NCU, CH), np.uint8)
        if M:
            gr = grank[c * DPC:(c + 1) * DPC]
            loc_inv = np.where(gr >= 0)[0]
            col_pos = {cb: i for i, cb in enumerate(cols_used)}
            for n in loc_inv:
                rr = int(gr[n])
                i = col_pos[n // 128]
                sel[rr % CHUNK,
                    ((rr // CHUNK) * NCU + i) * 128 + (n % 128)] = 1.0
            inv_full = (~valid).reshape(-1, 128).T
            for i, cb in enumerate(cols_used):
                invmask[:, i, :] = inv_full[:, cb][:, None]
        in_maps.append({
            **shared,
            "hidx": _wrap16(hidx), "sel": sel, "invmask": invmask,
        })
    return meta, in_maps


def _build_program(meta):
    M, NCHUNK, MP = meta["M"], meta["NCHUNK"], meta["MP"]
    TinvP = meta["TinvP"]
    n_lo, n_hi = meta["n_lo"], meta["n_hi"]
    SLP0, SLP1 = meta["SLP0"], meta["SLP1"]
    CMW = TinvP + 128 + CH + MP + N_REL * CH
    HALF = NCOL // 2                     # staging split for pipelining

    nc = bacc.Bacc("TRN2", target_bir_lowering=False, debug=False,
                   num_devices=N_CORES,
                   # all gathers together emit ~14k SWDGE descriptors; the
                   # default 1024-descriptor ring forces a mid-kernel drain
                   dynamic_dma_scratch_size=1 << 17)
    dt = mybir.dt
    d_tab0 = nc.dram_tensor("tab0", [T0_ROWS, CH], dt.float32, kind="ExternalInput")
    d_tab1 = nc.dram_tensor("tab1", [T1_ROWS, CH], dt.float32, kind="ExternalInput")
    d_ilo = nc.dram_tensor("idx_lo", [128, SLP0 // 16], dt.int16, kind="ExternalInput")
    d_ihi = nc.dram_tensor("idx_hi", [128, SLP1 // 16], dt.int16, kind="ExternalInput")
    d_cm = nc.dram_tensor("cmega", [128, CMW], dt.float32, kind="ExternalInput")
    d_hbuf = nc.dram_tensor("hbuf", [BUF, CH], dt.float32, kind="ExternalInput")
    d_hidx = nc.dram_tensor("hidx", [128, NPAD // 16], dt.int16, kind="ExternalInput")
    NCU = max(len(meta["cols_used"]), 1)
    d_sel = nc.dram_tensor("sel", [CHUNK, max(NCHUNK, 1) * NCU * 128],
                           dt.float32, kind="ExternalInput")
    d_invm = nc.dram_tensor("invmask", [128, NCU, CH], dt.uint8,
                            kind="ExternalInput")
    d_out = nc.dram_tensor("out", [128, NCOL, CH], dt.float32, kind="ExternalOutput")

    with tile.TileContext(nc) as tc:
        with (
            tc.tile_pool(name="const", bufs=1) as cpool,
            tc.tile_pool(name="g", bufs=2) as gpool,
            tc.tile_pool(name="s", bufs=2) as spool,
            tc.tile_pool(name="pz", bufs=2, space="PSUM") as pzpool,
            tc.tile_pool(name="po", bufs=2, space="PSUM") as popool,
            tc.tile_pool(name="pov", bufs=4, space="PSUM") as povpool,
        ):
            hidx_sb = cpool.tile([128, NPAD // 16], dt.int16)
            # two staging halves -> history gather and output DMA pipeline
            stages = [cpool.tile([128, HALF, CH], dt.float32, name="stageA"),
                      cpool.tile([128, NCOL - HALF, CH], dt.float32,
                                 name="stageB")]

            if M > 0:
                ilo_sb = cpool.tile([128, SLP0 // 16], dt.int16)
                ihi_sb = cpool.tile([128, SLP1 // 16], dt.int16)
                cm_sb = cpool.tile([128, CMW], dt.float32)
                sel_sb = cpool.tile([CHUNK, max(NCHUNK, 1) * NCU * 128],
                                    dt.float32)
                invm_sb = cpool.tile([128, NCU, CH], dt.uint8)
                # tiny inv idx arrays first so their gathers win the SWDGE
                # queue; the big history gather then fills the DMA engines
                # while the invalid-node rows are computed
                for i, (t_sb, t_d) in enumerate(
                        ((ilo_sb, d_ilo), (ihi_sb, d_ihi), (hidx_sb, d_hidx),
                         (cm_sb, d_cm), (sel_sb, d_sel), (invm_sb, d_invm))):
                    eng = nc.sync if i % 2 == 0 else nc.scalar
                    eng.dma_start(t_sb[:], t_d[:])
                o = 0
                srk_sb = cm_sb[:, 0:TinvP]; o = TinvP
                iota_sb = cm_sb[:, o:o + 128]; o += 128
                lwa_sb = cm_sb[:, o:o + CH]; o += CH
                xti_sb = cm_sb[:, o:o + MP]; o += MP
                wsb_o = o

                # per-(chunk, half) source-feature gathers
                gtiles = {}
                off = {0: 0, 1: 0}
                for ch in range(NCHUNK):
                    for h, n in ((0, int(n_lo[ch])), (1, int(n_hi[ch]))):
                        if n == 0:
                            continue
                        assert n * 128 <= BATCH
                        g = gpool.tile([128, n, CH], dt.float32, tag="g",
                                       name=f"g_{ch}_{h}")
                        tab = d_tab0 if h == 0 else d_tab1
                        isb = ilo_sb if h == 0 else ihi_sb
                        o2 = off[h]
                        nc.gpsimd.dma_gather(
                            g[:], tab[:], isb[:, o2 // 16:(o2 + n * 128) // 16],
                            num_idxs=n * 128, num_idxs_reg=n * 128,
                            elem_size=CH, single_packet=False,
                        )
                        gtiles[(ch, h)] = g
                        off[h] += n * 128

                gt = 0
                cps = []
                for ch in range(NCHUNK):
                    tl = meta["chunk_tiles"][ch]
                    ntot = len(tl)
                    if ntot:
                        pz = pzpool.tile([64, 128], dt.float32, tag="pz",
                                         name=f"pz_{ch}")
                        for i, (h, t) in enumerate(tl):
                            S = spool.tile([128, 128], dt.float32, tag="S",
                                           name=f"S_{ch}_{i}")
                            nc.vector.tensor_scalar(
                                S[:], iota_sb, srk_sb[:, gt:gt + 1], None,
                                mybir.AluOpType.is_equal,
                            )
                            xg = gtiles[(ch, h)][:, t, 0:CH]
                            nc.tensor.matmul(pz[:], xg, S[:],
                                             start=(i == 0),
                                             stop=(i == ntot - 1))
                            gt += 1
                        zt = spool.tile([64, 128], dt.float32, tag="zt",
                                        name=f"zt_{ch}")
                        nc.scalar.activation(zt[:], pz[:],
                                             mybir.ActivationFunctionType.Copy)
                    po = popool.tile([CHUNK, CH], dt.float32, tag="po",
                                     name=f"po_{ch}")
                    nc.tensor.matmul(po[:], xti_sb[:, ch * CHUNK:(ch + 1) * CHUNK],
                                     lwa_sb, start=True, stop=(ntot == 0))
                    if ntot:
                        for r in range(N_REL):
                            nc.tensor.matmul(
                                po[:], zt[:, r * CHUNK:(r + 1) * CHUNK],
                                cm_sb[0:64, wsb_o + r * CH:wsb_o + (r + 1) * CH],
                                start=False, stop=(r == N_REL - 1),
                            )
                    cp = cpool.tile([CHUNK, CH], dt.float32,
                                    name=f"cp_{ch}")
                    nc.vector.tensor_copy(cp[:], po[:])
                    cps.append(cp)

                # route computed rows to their positions; only columns that
                # hold an invalid node on some core need the overlay
                povs = []
                for i, cb in enumerate(meta["cols_used"]):
                    pov = povpool.tile([128, CH], dt.float32, tag="pov",
                                       name=f"pov_{cb}")
                    for ch in range(NCHUNK):
                        nc.tensor.matmul(
                            pov[:],
                            sel_sb[:, (ch * NCU + i) * 128:
                                   (ch * NCU + i) * 128 + 128],
                            cps[ch][:], start=(ch == 0),
                            stop=(ch == NCHUNK - 1),
                        )
                    povs.append(pov)

            if M == 0:
                nc.sync.dma_start(hidx_sb[:], d_hidx[:])
            # big history gathers issued after the (tiny) inv-compute DMAs so
            # the computed rows are ready the moment the history lands
            nidx = (HALF * 128, (NCOL - HALF) * 128)
            for half in range(2):
                o = half * HALF * 8      # idx cols consumed (128 idx / 8 col)
                nc.gpsimd.dma_gather(
                    stages[half][:], d_hbuf[:],
                    hidx_sb[:, o:o + nidx[half] // 16],
                    num_idxs=nidx[half], num_idxs_reg=nidx[half],
                    elem_size=CH, single_packet=False,
                )

            if M > 0:
                for i, cb in enumerate(meta["cols_used"]):
                    half, lc = (0, cb) if cb < HALF else (1, cb - HALF)
                    nc.vector.copy_predicated(stages[half][:, lc, :],
                                              invm_sb[:, i, :], povs[i][:])

            nc.sync.dma_start(d_out[:, 0:HALF, :], stages[0][:])
            nc.scalar.dma_start(d_out[:, HALF:NCOL, :], stages[1][:])
    nc.compile()
    return nc


def _prog_key(meta):
    return ("prog", meta["M"], meta["NCHUNK"], meta["Tinv"],
            tuple(meta["n_lo"]), tuple(meta["n_hi"]), meta["cols_used"])


def _run(inputs, trace=False):
    meta, in_maps = _host_prep(**inputs)
    key = _prog_key(meta)
    if key not in _cache:
        _cache[key] = _build_program(meta)
    nc = _cache[key]
    res = run_bass_kernel_spmd(nc, in_maps, list(range(N_CORES)), trace=trace)
    out = np.concatenate(
        [res.results[c]["out"].transpose(1, 0, 2).reshape(NPAD, CH)[:DPC]
         for c in range(N_CORES)], axis=0
    ).astype(np.float32)
    return out, res


def kernel(**inputs):
    out, _ = _run(inputs)
    return out


# revision 33
# speedup vs baseline: 2.1662x; 1.0244x over previous
"""RGCN-with-history (DGL RelGraphConv + history splice) on 8 TRN2 NeuronCores.

Key structural fact: the history splice dominates — out[n] is an exact copy of
history_buffer[history_map[n]] wherever history_map[n] >= 0, and the RGCN
aggregation only survives for the (very few) nodes with history_map[n] < 0.

Strategy (memory-bound regime):
  - Shard destination nodes across 8 cores (6250 each); each core
    indirect-gathers its history rows straight into two output staging
    halves (two dma_gathers, pipelined with the two output DMAs).
  - The globally-rare "no history" nodes are computed on every core
    (replicated tiny fp32 compute keeps the SPMD program identical): their
    incoming edges are bucketed into 16-node chunks; per 128-edge tile we
    indirect-gather source features and accumulate Z^T[64, 128] += Xg^T @ S
    on the tensor engine, where S is a (relation, node-rank) one-hot built
    on the vector engine (is_equal against an iota row). Relation weights +
    self-loop + bias are applied with small matmuls.
  - Computed rows are routed to their data-dependent positions with one-hot
    selector matmuls (only for the few staging columns that contain such a
    node on any core) and overlaid onto the history staging via predicated
    copies. Everything stays on-chip; no DRAM round-trip.
"""
import sys

sys.path.insert(0, "/opt/trn_rl_repo")

import numpy as np

import concourse.bacc as bacc
import concourse.tile as tile
import concourse.mybir as mybir
from concourse.bass_utils import run_bass_kernel_spmd

N_NODES = 50000
N_EDGES = 800000
CH = 64
N_REL = 8
BUF = 20000
N_CORES = 8
DPC = N_NODES // N_CORES            # 6250 dst nodes per core
NPAD = 6400                         # padded dst rows per core (50 x 128)
NCOL = NPAD // 128                  # 50 staging columns
SPLIT = 32767                       # src < SPLIT -> lo table, else hi
T0_ROWS = SPLIT + 1                 # lo table rows; row SPLIT is zeros
T1_ROWS = N_NODES - SPLIT + 1       # hi table rows; row 0 is zeros
CHUNK = 16                          # invalid nodes per compute chunk
BATCH = 4096                        # max gather indices per dma_gather

_cache = {}


def _wrap16(a):
    """Flat index array -> [128, len/16] int16 wrapped layout (idx k at
    [k%16, k//16], replicated across the 8 gpsimd lanes)."""
    m = a.reshape(-1, 16).T.astype(np.int16)
    return np.tile(m, (8, 1)).copy()


def _host_prep(x, W, loop_w, bias, history_buffer, src, dst, etypes, history_map):
    src = np.asarray(src)
    dst = np.asarray(dst)
    etypes = np.asarray(etypes)
    x = np.asarray(x, dtype=np.float32)
    hm = np.asarray(history_map)
    hb = np.asarray(history_buffer, np.float32)

    # --- shared gather tables (bf16, 128-col padded rows = 256B) ---
    tab0 = np.zeros((T0_ROWS, CH), np.float32)
    tab0[:SPLIT] = x[:SPLIT]
    tab1 = np.zeros((T1_ROWS, CH), np.float32)
    tab1[1:] = x[SPLIT:]

    # --- globally-rare invalid (no-history) nodes: replicated tiny compute ---
    inv_nodes = np.where(hm < 0)[0]              # sorted
    M = len(inv_nodes)
    NCHUNK = max(1, -(-M // CHUNK)) if M > 0 else 0
    MP = max(CHUNK, NCHUNK * CHUNK)              # scratch rows (>=16)

    n_lo = np.zeros(max(NCHUNK, 1), np.int64)
    n_hi = np.zeros(max(NCHUNK, 1), np.int64)
    idx_lo_slots = []
    idx_hi_slots = []
    srk_cols = None
    Tinv = 0
    chunk_tiles = []
    if M > 0:
        grank = np.full(N_NODES, -1, np.int64)
        grank[inv_nodes] = np.arange(M)
        emask = grank[dst] >= 0
        e_src = src[emask]
        e_et = etypes[emask]
        e_rank = grank[dst[emask]]
        e_chunk = e_rank // CHUNK
        e_half = (e_src >= SPLIT).astype(np.int64)
        e_col = e_et * CHUNK + (e_rank % CHUNK)  # one-hot col within chunk

        slots_lo = {}
        slots_hi = {}
        for ch in range(NCHUNK):
            for h, (slots, n_arr) in enumerate(((slots_lo, n_lo), (slots_hi, n_hi))):
                m = (e_chunk == ch) & (e_half == h)
                cnt = int(m.sum())
                n = -(-cnt // 128) if cnt else 0
                n_arr[ch] = n
                idxv = np.full(n * 128, SPLIT if h == 0 else 0, np.int64)
                srkv = np.zeros(n * 128, np.float32)
                s = e_src[m]
                idxv[:cnt] = s if h == 0 else s - (SPLIT - 1)
                srkv[:cnt] = e_col[m]
                slots[ch] = (idxv, srkv)
        # stream-concatenated slot arrays + srank columns in tile order
        srk_list = []
        for ch in range(NCHUNK):
            tl = []
            for h, slots in enumerate((slots_lo, slots_hi)):
                idxv, srkv = slots[ch]
                (idx_lo_slots if h == 0 else idx_hi_slots).append(idxv)
                for t in range(len(idxv) // 128):
                    srk_list.append(srkv[t * 128:(t + 1) * 128])
                    tl.append((h, t))
            chunk_tiles.append(tl)
        Tinv = len(srk_list)
        srk_cols = (np.stack(srk_list, axis=1) if Tinv
                    else np.zeros((128, 0), np.float32))

    SL0 = int(n_lo.sum()) * 128
    SL1 = int(n_hi.sum()) * 128
    SLP0 = max(128, SL0)
    SLP1 = max(128, SL1)
    idx_lo = np.full(SLP0, SPLIT, np.int64)
    idx_hi = np.zeros(SLP1, np.int64)
    if idx_lo_slots:
        cat = np.concatenate(idx_lo_slots) if SL0 else idx_lo[:0]
        idx_lo[:SL0] = cat
    if idx_hi_slots:
        cat = np.concatenate(idx_hi_slots) if SL1 else idx_hi[:0]
        idx_hi[:SL1] = cat
    TinvP = max(1, Tinv)
    srk = np.zeros((128, TinvP), np.float32)
    if Tinv:
        srk[:, :Tinv] = srk_cols

    # union (over cores) of staging columns that hold an invalid node —
    # only these columns need the computed-row overlay
    if M:
        inv_local = inv_nodes % DPC
        cols_used = sorted(set((inv_local // 128).tolist()))
    else:
        cols_used = []

    meta = {
        "M": M, "NCHUNK": NCHUNK, "MP": MP, "Tinv": Tinv, "TinvP": TinvP,
        "n_lo": n_lo, "n_hi": n_hi, "chunk_tiles": chunk_tiles,
        "SLP0": SLP0, "SLP1": SLP1, "cols_used": tuple(cols_used),
    }

    # --- weights / constants (shared) ---
    Wsb = np.zeros((64, N_REL, CH), np.float32)
    for r in range(N_REL):
        Wsb[:, r, :] = np.asarray(W[r], np.float32)
    lwa = np.zeros((128, CH), np.float32)
    lwa[:CH] = np.asarray(loop_w, np.float32)
    lwa[CH] = np.asarray(bias, np.float32)
    iota = np.tile(np.arange(128, dtype=np.float32)[None, :], (128, 1)).copy()
    xti = np.zeros((128, MP), np.float32)
    if M:
        xti[:CH, :M] = x[inv_nodes].T
        xti[CH, :M] = 1.0

    # merge the small f32 constants into one array (fewer DMAs):
    # [srk | iota(128) | lwa(64) | xti(MP) | wsb(512, rows 0:64)]
    cmega = np.zeros((128, TinvP + 128 + CH + MP + N_REL * CH), np.float32)
    o = 0
    cmega[:, o:o + TinvP] = srk; o += TinvP
    cmega[:, o:o + 128] = iota; o += 128
    cmega[:, o:o + CH] = lwa; o += CH
    cmega[:, o:o + MP] = xti; o += MP
    cmega[:64, o:o + N_REL * CH] = Wsb.reshape(64, N_REL * CH)

    shared = {
        "tab0": tab0, "tab1": tab1, "idx_lo": _wrap16(idx_lo),
        "idx_hi": _wrap16(idx_hi), "cmega": cmega, "hbuf": hb,
    }

    in_maps = []
    for c in range(N_CORES):
        hm_loc = np.zeros(NPAD, np.int64)
        hm_loc[:DPC] = hm[c * DPC:(c + 1) * DPC]
        hidx = np.clip(hm_loc, 0, BUF - 1)
        valid = hm_loc >= 0
        valid[DPC:] = True               # pad rows: treat as "history" side
        # selector + mask shipped only for the staging columns in cols_used
        NCU = max(len(cols_used), 1)
        sel = np.zeros((CHUNK, max(NCHUNK, 1) * NCU * 128), np.float32)
        invmask = np.zeros((128, NCU, CH), np.uint8)
        if M:
            gr = grank[c * DPC:(c + 1) * DPC]
            loc_inv = np.where(gr >= 0)[0]
            col_pos = {cb: i for i, cb in enumerate(cols_used)}
            for n in loc_inv:
                rr = int(gr[n])
                i = col_pos[n // 128]
                sel[rr % CHUNK,
                    ((rr // CHUNK) * NCU + i) * 128 + (n % 128)] = 1.0
            inv_full = (~valid).reshape(-1, 128).T
            for i, cb in enumerate(cols_used):
                invmask[:, i, :] = inv_full[:, cb][:, None]
        in_maps.append({
            **shared,
            "hidx": _wrap16(hidx), "sel": sel, "invmask": invmask,
        })
    return meta, in_maps


def _build_program(meta):
    M, NCHUNK, MP = meta["M"], meta["NCHUNK"], meta["MP"]
    TinvP = meta["TinvP"]
    n_lo, n_hi = meta["n_lo"], meta["n_hi"]
    SLP0, SLP1 = meta["SLP0"], meta["SLP1"]
    CMW = TinvP + 128 + CH + MP + N_REL * CH
    HALF = NCOL // 2                     # staging split for pipelining

    nc = bacc.Bacc("TRN2", target_bir_lowering=False, debug=False,
                   num_devices=N_CORES,
                   # all gathers together emit ~14k SWDGE descriptors; the
                   # default 1024-descriptor ring forces a mid-kernel drain
                   dynamic_dma_scratch_size=1 << 17)
    dt = mybir.dt
    d_tab0 = nc.dram_tensor("tab0", [T0_ROWS, CH], dt.float32, kind="ExternalInput")
    d_tab1 = nc.dram_tensor("tab1", [T1_ROWS, CH], dt.float32, kind="ExternalInput")
    d_ilo = nc.dram_tensor("idx_lo", [128, SLP0 // 16], dt.int16, kind="ExternalInput")
    d_ihi = nc.dram_tensor("idx_hi", [128, SLP1 // 16], dt.int16, kind="ExternalInput")
    d_cm = nc.dram_tensor("cmega", [128, CMW], dt.float32, kind="ExternalInput")
    d_hbuf = nc.dram_tensor("hbuf", [BUF, CH], dt.float32, kind="ExternalInput")
    d_hidx = nc.dram_tensor("hidx", [128, NPAD // 16], dt.int16, kind="ExternalInput")
    NCU = max(len(meta["cols_used"]), 1)
    d_sel = nc.dram_tensor("sel", [CHUNK, max(NCHUNK, 1) * NCU * 128],
                           dt.float32, kind="ExternalInput")
    d_invm = nc.dram_tensor("invmask", [128, NCU, CH], dt.uint8,
                            kind="ExternalInput")
    d_out = nc.dram_tensor("out", [128, NCOL, CH], dt.float32, kind="ExternalOutput")

    with tile.TileContext(nc) as tc:
        with (
            tc.tile_pool(name="const", bufs=1) as cpool,
            tc.tile_pool(name="g", bufs=2) as gpool,
            tc.tile_pool(name="s", bufs=2) as spool,
            tc.tile_pool(name="pz", bufs=2, space="PSUM") as pzpool,
            tc.tile_pool(name="po", bufs=2, space="PSUM") as popool,
            tc.tile_pool(name="pov", bufs=4, space="PSUM") as povpool,
        ):
            hidx_sb = cpool.tile([128, NPAD // 16], dt.int16)
            # two staging halves -> history gather and output DMA pipeline
            stages = [cpool.tile([128, HALF, CH], dt.float32, name="stageA"),
                      cpool.tile([128, NCOL - HALF, CH], dt.float32,
                                 name="stageB")]

            if M > 0:
                ilo_sb = cpool.tile([128, SLP0 // 16], dt.int16)
                ihi_sb = cpool.tile([128, SLP1 // 16], dt.int16)
                cm_sb = cpool.tile([128, CMW], dt.float32)
                sel_sb = cpool.tile([CHUNK, max(NCHUNK, 1) * NCU * 128],
                                    dt.float32)
                invm_sb = cpool.tile([128, NCU, CH], dt.uint8)
                # const DMA issue order controls which gathers win the SWDGE
                # queue and when history desc-gen can start (tuned against
                # the modeled timeline)
                for eng, pairs in (
                        (nc.sync, ((hidx_sb, d_hidx), (ilo_sb, d_ilo),
                                   (sel_sb, d_sel))),
                        (nc.scalar, ((ihi_sb, d_ihi), (cm_sb, d_cm),
                                     (invm_sb, d_invm)))):
                    for t_sb, t_d in pairs:
                        eng.dma_start(t_sb[:], t_d[:])
                o = 0
                srk_sb = cm_sb[:, 0:TinvP]; o = TinvP
                iota_sb = cm_sb[:, o:o + 128]; o += 128
                lwa_sb = cm_sb[:, o:o + CH]; o += CH
                xti_sb = cm_sb[:, o:o + MP]; o += MP
                wsb_o = o

                # per-(chunk, half) source-feature gathers
                gtiles = {}
                off = {0: 0, 1: 0}
                for ch in range(NCHUNK):
                    for h, n in ((0, int(n_lo[ch])), (1, int(n_hi[ch]))):
                        if n == 0:
                            continue
                        assert n * 128 <= BATCH
                        g = gpool.tile([128, n, CH], dt.float32, tag="g",
                                       name=f"g_{ch}_{h}")
                        tab = d_tab0 if h == 0 else d_tab1
                        isb = ilo_sb if h == 0 else ihi_sb
                        o2 = off[h]
                        nc.gpsimd.dma_gather(
                            g[:], tab[:], isb[:, o2 // 16:(o2 + n * 128) // 16],
                            num_idxs=n * 128, num_idxs_reg=n * 128,
                            elem_size=CH, single_packet=False,
                        )
                        gtiles[(ch, h)] = g
                        off[h] += n * 128

                gt = 0
                cps = []
                for ch in range(NCHUNK):
                    tl = meta["chunk_tiles"][ch]
                    ntot = len(tl)
                    if ntot:
                        pz = pzpool.tile([64, 128], dt.float32, tag="pz",
                                         name=f"pz_{ch}")
                        for i, (h, t) in enumerate(tl):
                            S = spool.tile([128, 128], dt.float32, tag="S",
                                           name=f"S_{ch}_{i}")
                            nc.vector.tensor_scalar(
                                S[:], iota_sb, srk_sb[:, gt:gt + 1], None,
                                mybir.AluOpType.is_equal,
                            )
                            xg = gtiles[(ch, h)][:, t, 0:CH]
                            nc.tensor.matmul(pz[:], xg, S[:],
                                             start=(i == 0),
                                             stop=(i == ntot - 1))
                            gt += 1
                        zt = spool.tile([64, 128], dt.float32, tag="zt",
                                        name=f"zt_{ch}")
                        nc.scalar.activation(zt[:], pz[:],
                                             mybir.ActivationFunctionType.Copy)
                    po = popool.tile([CHUNK, CH], dt.float32, tag="po",
                                     name=f"po_{ch}")
                    nc.tensor.matmul(po[:], xti_sb[:, ch * CHUNK:(ch + 1) * CHUNK],
                                     lwa_sb, start=True, stop=(ntot == 0))
                    if ntot:
                        for r in range(N_REL):
                            nc.tensor.matmul(
                                po[:], zt[:, r * CHUNK:(r + 1) * CHUNK],
                                cm_sb[0:64, wsb_o + r * CH:wsb_o + (r + 1) * CH],
                                start=False, stop=(r == N_REL - 1),
                            )
                    cp = cpool.tile([CHUNK, CH], dt.float32,
                                    name=f"cp_{ch}")
                    nc.vector.tensor_copy(cp[:], po[:])
                    cps.append(cp)

                # route computed rows to their positions; only columns that
                # hold an invalid node on some core need the overlay
                povs = []
                for i, cb in enumerate(meta["cols_used"]):
                    pov = povpool.tile([128, CH], dt.float32, tag="pov",
                                       name=f"pov_{cb}")
                    for ch in range(NCHUNK):
                        nc.tensor.matmul(
                            pov[:],
                            sel_sb[:, (ch * NCU + i) * 128:
                                   (ch * NCU + i) * 128 + 128],
                            cps[ch][:], start=(ch == 0),
                            stop=(ch == NCHUNK - 1),
                        )
                    povs.append(pov)

            if M == 0:
                nc.sync.dma_start(hidx_sb[:], d_hidx[:])
            # big history gathers issued after the (tiny) inv-compute DMAs so
            # the computed rows are ready the moment the history lands
            nidx = (HALF * 128, (NCOL - HALF) * 128)
            for half in range(2):
                o = half * HALF * 8      # idx cols consumed (128 idx / 8 col)
                nc.gpsimd.dma_gather(
                    stages[half][:], d_hbuf[:],
                    hidx_sb[:, o:o + nidx[half] // 16],
                    num_idxs=nidx[half], num_idxs_reg=nidx[half],
                    elem_size=CH, single_packet=False,
                )

            if M > 0:
                for i, cb in enumerate(meta["cols_used"]):
                    half, lc = (0, cb) if cb < HALF else (1, cb - HALF)
                    nc.vector.copy_predicated(stages[half][:, lc, :],
                                              invm_sb[:, i, :], povs[i][:])

            nc.sync.dma_start(d_out[:, 0:HALF, :], stages[0][:])
            nc.scalar.dma_start(d_out[:, HALF:NCOL, :], stages[1][:])
    nc.compile()
    return nc


def _prog_key(meta):
    return ("prog", meta["M"], meta["NCHUNK"], meta["Tinv"],
            tuple(meta["n_lo"]), tuple(meta["n_hi"]), meta["cols_used"])


def _run(inputs, trace=False):
    meta, in_maps = _host_prep(**inputs)
    key = _prog_key(meta)
    if key not in _cache:
        _cache[key] = _build_program(meta)
    nc = _cache[key]
    res = run_bass_kernel_spmd(nc, in_maps, list(range(N_CORES)), trace=trace)
    out = np.concatenate(
        [res.results[c]["out"].transpose(1, 0, 2).reshape(NPAD, CH)[:DPC]
         for c in range(N_CORES)], axis=0
    ).astype(np.float32)
    return out, res


def kernel(**inputs):
    out, _ = _run(inputs)
    return out


# revision 36
# speedup vs baseline: 2.2932x; 1.0586x over previous
"""RGCN-with-history (DGL RelGraphConv + history splice) on 8 TRN2 NeuronCores.

Key structural fact: the history splice dominates — out[n] is an exact copy of
history_buffer[history_map[n]] wherever history_map[n] >= 0, and the RGCN
aggregation only survives for the (very few) nodes with history_map[n] < 0.

Strategy (memory-bound regime):
  - Shard destination nodes across 8 cores (6250 each); each core
    indirect-gathers its history rows straight into two output staging
    halves (two dma_gathers, pipelined with the two output DMAs).
  - The globally-rare "no history" nodes are computed on every core
    (replicated tiny fp32 compute keeps the SPMD program identical): their
    incoming edges are bucketed into 16-node chunks; per 128-edge tile we
    indirect-gather source features and accumulate Z^T[64, 128] += Xg^T @ S
    on the tensor engine, where S is a (relation, node-rank) one-hot built
    on the vector engine (is_equal against an iota row). Relation weights +
    self-loop + bias are applied with small matmuls.
  - Computed rows are routed to their data-dependent positions with one-hot
    selector matmuls (only for the few staging columns that contain such a
    node on any core) and overlaid onto the history staging via predicated
    copies. Everything stays on-chip; no DRAM round-trip.
"""
import sys

sys.path.insert(0, "/opt/trn_rl_repo")

import numpy as np

import concourse.bacc as bacc
import concourse.tile as tile
import concourse.mybir as mybir
from concourse.bass_utils import run_bass_kernel_spmd

N_NODES = 50000
N_EDGES = 800000
CH = 64
N_REL = 8
BUF = 20000
N_CORES = 8
DPC = N_NODES // N_CORES            # 6250 dst nodes per core
NPAD = 6400                         # padded dst rows per core (50 x 128)
NCOL = NPAD // 128                  # 50 staging columns
SPLIT = 32767                       # src < SPLIT -> lo table, else hi
T0_ROWS = SPLIT + 1                 # lo table rows; row SPLIT is zeros
T1_ROWS = N_NODES - SPLIT + 1       # hi table rows; row 0 is zeros
CHUNK = 16                          # invalid nodes per compute chunk
BATCH = 4096                        # max gather indices per dma_gather

_cache = {}


def _wrap16(a):
    """Flat index array -> [128, len/16] int16 wrapped layout (idx k at
    [k%16, k//16], replicated across the 8 gpsimd lanes)."""
    m = a.reshape(-1, 16).T.astype(np.int16)
    return np.tile(m, (8, 1)).copy()


def _host_prep(x, W, loop_w, bias, history_buffer, src, dst, etypes, history_map):
    src = np.asarray(src)
    dst = np.asarray(dst)
    etypes = np.asarray(etypes)
    x = np.asarray(x, dtype=np.float32)
    hm = np.asarray(history_map)
    hb = np.asarray(history_buffer, np.float32)

    # --- globally-rare invalid (no-history) nodes: replicated tiny compute ---
    inv_nodes = np.where(hm < 0)[0]              # sorted
    M = len(inv_nodes)
    NCHUNK = max(1, -(-M // CHUNK)) if M > 0 else 0
    MP = max(CHUNK, NCHUNK * CHUNK)              # scratch rows (>=16)

    n_lo = np.zeros(max(NCHUNK, 1), np.int64)
    n_hi = np.zeros(max(NCHUNK, 1), np.int64)
    idx_lo_slots = []
    idx_hi_slots = []
    srk_cols = None
    Tinv = 0
    chunk_tiles = []
    if M > 0:
        grank = np.full(N_NODES, -1, np.int64)
        grank[inv_nodes] = np.arange(M)
        emask = grank[dst] >= 0
        e_src = src[emask]
        e_et = etypes[emask]
        e_rank = grank[dst[emask]]
        e_chunk = e_rank // CHUNK
        e_half = (e_src >= SPLIT).astype(np.int64)
        e_col = e_et * CHUNK + (e_rank % CHUNK)  # one-hot col within chunk

        # host-side halo of the invalid edges' source features (the
        # sharding hint's "halo of remote source features"): per 128-edge
        # tile, a [128, CH] f32 block; pad edges are zero rows.
        srk_list = []
        xg_list = []
        for ch in range(NCHUNK):
            m = e_chunk == ch
            cnt = int(m.sum())
            n = -(-cnt // 128) if cnt else 0
            n_lo[ch] = n
            srkv = np.zeros(n * 128, np.float32)
            srkv[:cnt] = e_col[m]
            xgv = np.zeros((n * 128, CH), np.float32)
            xgv[:cnt] = x[e_src[m]]
            tl = []
            for t in range(n):
                srk_list.append(srkv[t * 128:(t + 1) * 128])
                xg_list.append(xgv[t * 128:(t + 1) * 128])
                tl.append((0, t))
            chunk_tiles.append(tl)
        Tinv = len(srk_list)
        srk_cols = (np.stack(srk_list, axis=1) if Tinv
                    else np.zeros((128, 0), np.float32))

    TinvP = max(1, Tinv)
    srk = np.zeros((128, TinvP), np.float32)
    xg_halo = np.zeros((128, TinvP, CH), np.float32)
    if Tinv:
        srk[:, :Tinv] = srk_cols
        for t, blk in enumerate(xg_list):
            xg_halo[:, t, :] = blk

    # union (over cores) of staging columns that hold an invalid node —
    # only these columns need the computed-row overlay
    if M:
        inv_local = inv_nodes % DPC
        cols_used = sorted(set((inv_local // 128).tolist()))
    else:
        cols_used = []

    meta = {
        "M": M, "NCHUNK": NCHUNK, "MP": MP, "Tinv": Tinv, "TinvP": TinvP,
        "n_lo": n_lo, "n_hi": n_hi, "chunk_tiles": chunk_tiles,
        "cols_used": tuple(cols_used),
    }

    # --- weights / constants (shared) ---
    Wsb = np.zeros((64, N_REL, CH), np.float32)
    for r in range(N_REL):
        Wsb[:, r, :] = np.asarray(W[r], np.float32)
    lwa = np.zeros((128, CH), np.float32)
    lwa[:CH] = np.asarray(loop_w, np.float32)
    lwa[CH] = np.asarray(bias, np.float32)
    iota = np.tile(np.arange(128, dtype=np.float32)[None, :], (128, 1)).copy()
    xti = np.zeros((128, MP), np.float32)
    if M:
        xti[:CH, :M] = x[inv_nodes].T
        xti[CH, :M] = 1.0

    # merge the small f32 constants into one array (fewer DMAs):
    # [srk | iota(128) | lwa(64) | xti(MP) | wsb(512, rows 0:64)]
    cmega = np.zeros((128, TinvP + 128 + CH + MP + N_REL * CH), np.float32)
    o = 0
    cmega[:, o:o + TinvP] = srk; o += TinvP
    cmega[:, o:o + 128] = iota; o += 128
    cmega[:, o:o + CH] = lwa; o += CH
    cmega[:, o:o + MP] = xti; o += MP
    cmega[:64, o:o + N_REL * CH] = Wsb.reshape(64, N_REL * CH)

    shared = {"cmega": cmega, "xg": xg_halo, "hbuf": hb}

    in_maps = []
    for c in range(N_CORES):
        hm_loc = np.zeros(NPAD, np.int64)
        hm_loc[:DPC] = hm[c * DPC:(c + 1) * DPC]
        hidx = np.clip(hm_loc, 0, BUF - 1)
        valid = hm_loc >= 0
        valid[DPC:] = True               # pad rows: treat as "history" side
        # selector + mask shipped only for the staging columns in cols_used
        NCU = max(len(cols_used), 1)
        sel = np.zeros((CHUNK, max(NCHUNK, 1) * NCU * 128), np.float32)
        invmask = np.zeros((128, NCU, CH), np.uint8)
        if M:
            gr = grank[c * DPC:(c + 1) * DPC]
            loc_inv = np.where(gr >= 0)[0]
            col_pos = {cb: i for i, cb in enumerate(cols_used)}
            for n in loc_inv:
                rr = int(gr[n])
                i = col_pos[n // 128]
                sel[rr % CHUNK,
                    ((rr // CHUNK) * NCU + i) * 128 + (n % 128)] = 1.0
            inv_full = (~valid).reshape(-1, 128).T
            for i, cb in enumerate(cols_used):
                invmask[:, i, :] = inv_full[:, cb][:, None]
        in_maps.append({
            **shared,
            "hidx": _wrap16(hidx), "sel": sel, "invmask": invmask,
        })
    return meta, in_maps


def _build_program(meta):
    M, NCHUNK, MP = meta["M"], meta["NCHUNK"], meta["MP"]
    TinvP = meta["TinvP"]
    CMW = TinvP + 128 + CH + MP + N_REL * CH
    HALF = NCOL // 2                     # staging split for pipelining

    nc = bacc.Bacc("TRN2", target_bir_lowering=False, debug=False,
                   num_devices=N_CORES,
                   # all gathers together emit ~14k SWDGE descriptors; the
                   # default 1024-descriptor ring forces a mid-kernel drain
                   dynamic_dma_scratch_size=1 << 17)
    dt = mybir.dt
    d_cm = nc.dram_tensor("cmega", [128, CMW], dt.float32, kind="ExternalInput")
    d_xg = nc.dram_tensor("xg", [128, TinvP, CH], dt.float32,
                          kind="ExternalInput")
    d_hbuf = nc.dram_tensor("hbuf", [BUF, CH], dt.float32, kind="ExternalInput")
    d_hidx = nc.dram_tensor("hidx", [128, NPAD // 16], dt.int16, kind="ExternalInput")
    NCU = max(len(meta["cols_used"]), 1)
    d_sel = nc.dram_tensor("sel", [CHUNK, max(NCHUNK, 1) * NCU * 128],
                           dt.float32, kind="ExternalInput")
    d_invm = nc.dram_tensor("invmask", [128, NCU, CH], dt.uint8,
                            kind="ExternalInput")
    d_out = nc.dram_tensor("out", [128, NCOL, CH], dt.float32, kind="ExternalOutput")

    with tile.TileContext(nc) as tc:
        with (
            tc.tile_pool(name="const", bufs=1) as cpool,
            tc.tile_pool(name="g", bufs=2) as gpool,
            tc.tile_pool(name="s", bufs=2) as spool,
            tc.tile_pool(name="pz", bufs=2, space="PSUM") as pzpool,
            tc.tile_pool(name="po", bufs=2, space="PSUM") as popool,
            tc.tile_pool(name="pov", bufs=4, space="PSUM") as povpool,
        ):
            hidx_sb = cpool.tile([128, NPAD // 16], dt.int16)
            # two staging halves -> history gather and output DMA pipeline
            stages = [cpool.tile([128, HALF, CH], dt.float32, name="stageA"),
                      cpool.tile([128, NCOL - HALF, CH], dt.float32,
                                 name="stageB")]

            if M > 0:
                xg_sb = cpool.tile([128, TinvP, CH], dt.float32)
                cm_sb = cpool.tile([128, CMW], dt.float32)
                sel_sb = cpool.tile([CHUNK, max(NCHUNK, 1) * NCU * 128],
                                    dt.float32)
                invm_sb = cpool.tile([128, NCU, CH], dt.uint8)
                # const DMA issue order controls when history desc-gen can
                # start (hidx first) vs. when the invalid-node compute chain
                # has its operands (tuned against the modeled timeline)
                for eng, pairs in (
                        (nc.sync, ((hidx_sb, d_hidx), (xg_sb, d_xg),
                                   (sel_sb, d_sel))),
                        (nc.scalar, ((cm_sb, d_cm), (invm_sb, d_invm)))):
                    for t_sb, t_d in pairs:
                        eng.dma_start(t_sb[:], t_d[:])
                o = 0
                srk_sb = cm_sb[:, 0:TinvP]; o = TinvP
                iota_sb = cm_sb[:, o:o + 128]; o += 128
                lwa_sb = cm_sb[:, o:o + CH]; o += CH
                xti_sb = cm_sb[:, o:o + MP]; o += MP
                wsb_o = o

                gt = 0
                cps = []
                for ch in range(NCHUNK):
                    tl = meta["chunk_tiles"][ch]
                    ntot = len(tl)
                    if ntot:
                        pz = pzpool.tile([64, 128], dt.float32, tag="pz",
                                         name=f"pz_{ch}")
                        for i, (h, t) in enumerate(tl):
                            S = spool.tile([128, 128], dt.float32, tag="S",
                                           name=f"S_{ch}_{i}")
                            nc.vector.tensor_scalar(
                                S[:], iota_sb, srk_sb[:, gt:gt + 1], None,
                                mybir.AluOpType.is_equal,
                            )
                            nc.tensor.matmul(pz[:], xg_sb[:, gt, :], S[:],
                                             start=(i == 0),
                                             stop=(i == ntot - 1))
                            gt += 1
                        zt = spool.tile([64, 128], dt.float32, tag="zt",
                                        name=f"zt_{ch}")
                        nc.scalar.activation(zt[:], pz[:],
                                             mybir.ActivationFunctionType.Copy)
                    po = popool.tile([CHUNK, CH], dt.float32, tag="po",
                                     name=f"po_{ch}")
                    nc.tensor.matmul(po[:], xti_sb[:, ch * CHUNK:(ch + 1) * CHUNK],
                                     lwa_sb, start=True, stop=(ntot == 0))
                    if ntot:
                        for r in range(N_REL):
                            nc.tensor.matmul(
                                po[:], zt[:, r * CHUNK:(r + 1) * CHUNK],
                                cm_sb[0:64, wsb_o + r * CH:wsb_o + (r + 1) * CH],
                                start=False, stop=(r == N_REL - 1),
                            )
                    cp = cpool.tile([CHUNK, CH], dt.float32,
                                    name=f"cp_{ch}")
                    nc.vector.tensor_copy(cp[:], po[:])
                    cps.append(cp)

                # route computed rows to their positions; only columns that
                # hold an invalid node on some core need the overlay
                povs = []
                for i, cb in enumerate(meta["cols_used"]):
                    pov = povpool.tile([128, CH], dt.float32, tag="pov",
                                       name=f"pov_{cb}")
                    for ch in range(NCHUNK):
                        nc.tensor.matmul(
                            pov[:],
                            sel_sb[:, (ch * NCU + i) * 128:
                                   (ch * NCU + i) * 128 + 128],
                            cps[ch][:], start=(ch == 0),
                            stop=(ch == NCHUNK - 1),
                        )
                    povs.append(pov)

            if M == 0:
                nc.sync.dma_start(hidx_sb[:], d_hidx[:])
            # big history gathers issued after the (tiny) inv-compute DMAs so
            # the computed rows are ready the moment the history lands
            nidx = (HALF * 128, (NCOL - HALF) * 128)
            for half in range(2):
                o = half * HALF * 8      # idx cols consumed (128 idx / 8 col)
                nc.gpsimd.dma_gather(
                    stages[half][:], d_hbuf[:],
                    hidx_sb[:, o:o + nidx[half] // 16],
                    num_idxs=nidx[half], num_idxs_reg=nidx[half],
                    elem_size=CH, single_packet=False,
                )

            if M > 0:
                for i, cb in enumerate(meta["cols_used"]):
                    half, lc = (0, cb) if cb < HALF else (1, cb - HALF)
                    nc.vector.copy_predicated(stages[half][:, lc, :],
                                              invm_sb[:, i, :], povs[i][:])

            nc.sync.dma_start(d_out[:, 0:HALF, :], stages[0][:])
            nc.scalar.dma_start(d_out[:, HALF:NCOL, :], stages[1][:])
    nc.compile()
    return nc


def _prog_key(meta):
    return ("prog", meta["M"], meta["NCHUNK"], meta["Tinv"],
            tuple(meta["n_lo"]), tuple(meta["n_hi"]), meta["cols_used"])


def _run(inputs, trace=False):
    meta, in_maps = _host_prep(**inputs)
    key = _prog_key(meta)
    if key not in _cache:
        _cache[key] = _build_program(meta)
    nc = _cache[key]
    res = run_bass_kernel_spmd(nc, in_maps, list(range(N_CORES)), trace=trace)
    out = np.concatenate(
        [res.results[c]["out"].transpose(1, 0, 2).reshape(NPAD, CH)[:DPC]
         for c in range(N_CORES)], axis=0
    ).astype(np.float32)
    return out, res


def kernel(**inputs):
    out, _ = _run(inputs)
    return out


# revision 39
# speedup vs baseline: 2.3539x; 1.0265x over previous
"""RGCN-with-history (DGL RelGraphConv + history splice) on 8 TRN2 NeuronCores.

Key structural fact: the history splice dominates — out[n] is an exact copy of
history_buffer[history_map[n]] wherever history_map[n] >= 0, and the RGCN
aggregation only survives for the (very few) nodes with history_map[n] < 0.

Strategy (memory-bound regime):
  - Shard destination nodes across 8 cores (6250 each); each core
    indirect-gathers its history rows straight into two output staging
    halves (two dma_gathers, pipelined with the two output DMAs).
  - The globally-rare "no history" nodes are computed on every core
    (replicated tiny fp32 compute keeps the SPMD program identical): their
    incoming edges are bucketed into 16-node chunks; per 128-edge tile we
    indirect-gather source features and accumulate Z^T[64, 128] += Xg^T @ S
    on the tensor engine, where S is a (relation, node-rank) one-hot built
    on the vector engine (is_equal against an iota row). Relation weights +
    self-loop + bias are applied with small matmuls.
  - Computed rows are routed to their data-dependent positions with one-hot
    selector matmuls (only for the few staging columns that contain such a
    node on any core) and overlaid onto the history staging via predicated
    copies. Everything stays on-chip; no DRAM round-trip.
"""
import sys

sys.path.insert(0, "/opt/trn_rl_repo")

import numpy as np

import concourse.bacc as bacc
import concourse.tile as tile
import concourse.mybir as mybir
from concourse.bass_utils import run_bass_kernel_spmd

N_NODES = 50000
N_EDGES = 800000
CH = 64
N_REL = 8
BUF = 20000
N_CORES = 8
DPC = N_NODES // N_CORES            # 6250 dst nodes per core
NPAD = 6400                         # padded dst rows per core (50 x 128)
NCOL = NPAD // 128                  # 50 staging columns
SPLIT = 32767                       # src < SPLIT -> lo table, else hi
T0_ROWS = SPLIT + 1                 # lo table rows; row SPLIT is zeros
T1_ROWS = N_NODES - SPLIT + 1       # hi table rows; row 0 is zeros
CHUNK = 16                          # invalid nodes per compute chunk
BATCH = 4096                        # max gather indices per dma_gather

_cache = {}


def _wrap16(a):
    """Flat index array -> [128, len/16] int16 wrapped layout (idx k at
    [k%16, k//16], replicated across the 8 gpsimd lanes)."""
    m = a.reshape(-1, 16).T.astype(np.int16)
    return np.tile(m, (8, 1)).copy()


def _host_prep(x, W, loop_w, bias, history_buffer, src, dst, etypes, history_map):
    src = np.asarray(src)
    dst = np.asarray(dst)
    etypes = np.asarray(etypes)
    x = np.asarray(x, dtype=np.float32)
    hm = np.asarray(history_map)
    hb = np.asarray(history_buffer, np.float32)

    # --- globally-rare invalid (no-history) nodes: replicated tiny compute ---
    inv_nodes = np.where(hm < 0)[0]              # sorted
    M = len(inv_nodes)
    NCHUNK = max(1, -(-M // CHUNK)) if M > 0 else 0
    MP = max(CHUNK, NCHUNK * CHUNK)              # scratch rows (>=16)

    n_lo = np.zeros(max(NCHUNK, 1), np.int64)
    n_hi = np.zeros(max(NCHUNK, 1), np.int64)
    idx_lo_slots = []
    idx_hi_slots = []
    srk_cols = None
    Tinv = 0
    chunk_tiles = []
    if M > 0:
        grank = np.full(N_NODES, -1, np.int64)
        grank[inv_nodes] = np.arange(M)
        emask = grank[dst] >= 0
        e_src = src[emask]
        e_et = etypes[emask]
        e_rank = grank[dst[emask]]
        e_chunk = e_rank // CHUNK
        e_half = (e_src >= SPLIT).astype(np.int64)
        e_col = e_et * CHUNK + (e_rank % CHUNK)  # one-hot col within chunk

        # host-side halo of the invalid edges' source features (the
        # sharding hint's "halo of remote source features"): per 128-edge
        # tile, a [128, CH] f32 block; pad edges are zero rows.
        srk_list = []
        xg_list = []
        for ch in range(NCHUNK):
            m = e_chunk == ch
            cnt = int(m.sum())
            n = -(-cnt // 128) if cnt else 0
            n_lo[ch] = n
            srkv = np.zeros(n * 128, np.float32)
            srkv[:cnt] = e_col[m]
            xgv = np.zeros((n * 128, CH), np.float32)
            xgv[:cnt] = x[e_src[m]]
            tl = []
            for t in range(n):
                srk_list.append(srkv[t * 128:(t + 1) * 128])
                xg_list.append(xgv[t * 128:(t + 1) * 128])
                tl.append((0, t))
            chunk_tiles.append(tl)
        Tinv = len(srk_list)
        srk_cols = (np.stack(srk_list, axis=1) if Tinv
                    else np.zeros((128, 0), np.float32))

    TinvP = max(1, Tinv)
    srk = np.zeros((128, TinvP), np.float32)
    xg_halo = np.zeros((128, TinvP, CH), np.float32)
    if Tinv:
        srk[:, :Tinv] = srk_cols
        for t, blk in enumerate(xg_list):
            xg_halo[:, t, :] = blk

    # union (over cores) of staging columns that hold an invalid node —
    # only these columns need the computed-row overlay
    if M:
        inv_local = inv_nodes % DPC
        cols_used = sorted(set((inv_local // 128).tolist()))
    else:
        cols_used = []

    meta = {
        "M": M, "NCHUNK": NCHUNK, "MP": MP, "Tinv": Tinv, "TinvP": TinvP,
        "n_lo": n_lo, "n_hi": n_hi, "chunk_tiles": chunk_tiles,
        "cols_used": tuple(cols_used),
    }

    # --- weights / constants (shared) ---
    Wsb = np.zeros((64, N_REL, CH), np.float32)
    for r in range(N_REL):
        Wsb[:, r, :] = np.asarray(W[r], np.float32)
    lwa = np.zeros((128, CH), np.float32)
    lwa[:CH] = np.asarray(loop_w, np.float32)
    lwa[CH] = np.asarray(bias, np.float32)
    iota = np.tile(np.arange(128, dtype=np.float32)[None, :], (128, 1)).copy()
    xti = np.zeros((128, MP), np.float32)
    if M:
        xti[:CH, :M] = x[inv_nodes].T
        xti[CH, :M] = 1.0

    # merge the small f32 constants into one array (fewer DMAs):
    # [srk | iota(128) | lwa(64) | xti(MP) | wsb(512, rows 0:64)]
    cmega = np.zeros((128, TinvP + 128 + CH + MP + N_REL * CH), np.float32)
    o = 0
    cmega[:, o:o + TinvP] = srk; o += TinvP
    cmega[:, o:o + 128] = iota; o += 128
    cmega[:, o:o + CH] = lwa; o += CH
    cmega[:, o:o + MP] = xti; o += MP
    cmega[:64, o:o + N_REL * CH] = Wsb.reshape(64, N_REL * CH)

    shared = {"cmega": cmega, "xg": xg_halo, "hbuf": hb}

    in_maps = []
    for c in range(N_CORES):
        hm_loc = np.zeros(NPAD, np.int64)
        hm_loc[:DPC] = hm[c * DPC:(c + 1) * DPC]
        hidx = np.clip(hm_loc, 0, BUF - 1)
        valid = hm_loc >= 0
        valid[DPC:] = True               # pad rows: treat as "history" side
        # selector + mask shipped only for the staging columns in cols_used
        NCU = max(len(cols_used), 1)
        sel = np.zeros((CHUNK, max(NCHUNK, 1) * NCU * 128), np.float32)
        invmask = np.zeros((128, NCU, CH), np.uint8)
        if M:
            gr = grank[c * DPC:(c + 1) * DPC]
            loc_inv = np.where(gr >= 0)[0]
            col_pos = {cb: i for i, cb in enumerate(cols_used)}
            for n in loc_inv:
                rr = int(gr[n])
                i = col_pos[n // 128]
                sel[rr % CHUNK,
                    ((rr // CHUNK) * NCU + i) * 128 + (n % 128)] = 1.0
            inv_full = (~valid).reshape(-1, 128).T
            for i, cb in enumerate(cols_used):
                invmask[:, i, :] = inv_full[:, cb][:, None]
        in_maps.append({
            **shared,
            "hidx": _wrap16(hidx), "sel": sel, "invmask": invmask,
        })
    return meta, in_maps


def _build_program(meta):
    M, NCHUNK, MP = meta["M"], meta["NCHUNK"], meta["MP"]
    TinvP = meta["TinvP"]
    CMW = TinvP + 128 + CH + MP + N_REL * CH
    HALF = NCOL // 2                     # staging split for pipelining

    nc = bacc.Bacc("TRN2", target_bir_lowering=False, debug=False,
                   num_devices=N_CORES,
                   # all gathers together emit ~14k SWDGE descriptors; the
                   # default 1024-descriptor ring forces a mid-kernel drain
                   dynamic_dma_scratch_size=1 << 17)
    dt = mybir.dt
    d_cm = nc.dram_tensor("cmega", [128, CMW], dt.float32, kind="ExternalInput")
    d_xg = nc.dram_tensor("xg", [128, TinvP, CH], dt.float32,
                          kind="ExternalInput")
    d_hbuf = nc.dram_tensor("hbuf", [BUF, CH], dt.float32, kind="ExternalInput")
    d_hidx = nc.dram_tensor("hidx", [128, NPAD // 16], dt.int16, kind="ExternalInput")
    NCU = max(len(meta["cols_used"]), 1)
    d_sel = nc.dram_tensor("sel", [CHUNK, max(NCHUNK, 1) * NCU * 128],
                           dt.float32, kind="ExternalInput")
    d_invm = nc.dram_tensor("invmask", [128, NCU, CH], dt.uint8,
                            kind="ExternalInput")
    d_out = nc.dram_tensor("out", [128, NCOL, CH], dt.float32, kind="ExternalOutput")

    with tile.TileContext(nc) as tc:
        with (
            tc.tile_pool(name="const", bufs=1) as cpool,
            tc.tile_pool(name="g", bufs=2) as gpool,
            tc.tile_pool(name="s", bufs=2) as spool,
            tc.tile_pool(name="pz", bufs=2, space="PSUM") as pzpool,
            tc.tile_pool(name="po", bufs=2, space="PSUM") as popool,
            tc.tile_pool(name="pov", bufs=4, space="PSUM") as povpool,
        ):
            hidx_sb = cpool.tile([128, NPAD // 16], dt.int16)
            # two staging halves -> history gather and output DMA pipeline
            stages = [cpool.tile([128, HALF, CH], dt.float32, name="stageA"),
                      cpool.tile([128, NCOL - HALF, CH], dt.float32,
                                 name="stageB")]

            if M > 0:
                xg_sb = cpool.tile([128, TinvP, CH], dt.float32)
                cm_sb = cpool.tile([128, CMW], dt.float32)
                sel_sb = cpool.tile([CHUNK, max(NCHUNK, 1) * NCU * 128],
                                    dt.float32)
                invm_sb = cpool.tile([128, NCU, CH], dt.uint8)
                # const DMA issue order controls when history desc-gen can
                # start (hidx first) vs. when the invalid-node compute chain
                # has its operands (tuned against the modeled timeline)
                for eng, pairs in (
                        (nc.sync, ((hidx_sb, d_hidx), (xg_sb, d_xg),
                                   (sel_sb, d_sel))),
                        (nc.scalar, ((cm_sb, d_cm), (invm_sb, d_invm)))):
                    for t_sb, t_d in pairs:
                        eng.dma_start(t_sb[:], t_d[:])
                o = 0
                srk_sb = cm_sb[:, 0:TinvP]; o = TinvP
                iota_sb = cm_sb[:, o:o + 128]; o += 128
                lwa_sb = cm_sb[:, o:o + CH]; o += CH
                xti_sb = cm_sb[:, o:o + MP]; o += MP
                wsb_o = o

                gt = 0
                cps = []
                for ch in range(NCHUNK):
                    tl = meta["chunk_tiles"][ch]
                    ntot = len(tl)
                    if ntot:
                        pz = pzpool.tile([64, 128], dt.float32, tag="pz",
                                         name=f"pz_{ch}")
                        for i, (h, t) in enumerate(tl):
                            S = spool.tile([128, 128], dt.float32, tag="S",
                                           name=f"S_{ch}_{i}")
                            nc.vector.tensor_scalar(
                                S[:], iota_sb, srk_sb[:, gt:gt + 1], None,
                                mybir.AluOpType.is_equal,
                            )
                            nc.tensor.matmul(pz[:], xg_sb[:, gt, :], S[:],
                                             start=(i == 0),
                                             stop=(i == ntot - 1))
                            gt += 1
                        zt = spool.tile([64, 128], dt.float32, tag="zt",
                                        name=f"zt_{ch}")
                        nc.scalar.activation(zt[:], pz[:],
                                             mybir.ActivationFunctionType.Copy)
                    po = popool.tile([CHUNK, CH], dt.float32, tag="po",
                                     name=f"po_{ch}")
                    nc.tensor.matmul(po[:], xti_sb[:, ch * CHUNK:(ch + 1) * CHUNK],
                                     lwa_sb, start=True, stop=(ntot == 0))
                    if ntot:
                        for r in range(N_REL):
                            nc.tensor.matmul(
                                po[:], zt[:, r * CHUNK:(r + 1) * CHUNK],
                                cm_sb[0:64, wsb_o + r * CH:wsb_o + (r + 1) * CH],
                                start=False, stop=(r == N_REL - 1),
                            )
                    cp = cpool.tile([CHUNK, CH], dt.float32,
                                    name=f"cp_{ch}")
                    nc.vector.tensor_copy(cp[:], po[:])
                    cps.append(cp)

                # route computed rows to their positions; only columns that
                # hold an invalid node on some core need the overlay
                povs = []
                for i, cb in enumerate(meta["cols_used"]):
                    pov = povpool.tile([128, CH], dt.float32, tag="pov",
                                       name=f"pov_{cb}")
                    for ch in range(NCHUNK):
                        nc.tensor.matmul(
                            pov[:],
                            sel_sb[:, (ch * NCU + i) * 128:
                                   (ch * NCU + i) * 128 + 128],
                            cps[ch][:], start=(ch == 0),
                            stop=(ch == NCHUNK - 1),
                        )
                    povs.append(pov)

            if M == 0:
                nc.sync.dma_start(hidx_sb[:], d_hidx[:])
            # history gathers: a small head segment first so its (short)
            # desc-gen completes early and transfers start sooner; later
            # segments' desc-gen pipelines behind running transfers
            segs = ((0, 0, 12), (0, 12, HALF - 12), (1, 0, NCOL - HALF))
            o8 = 0
            for st, co, ncols in segs:
                ni = ncols * 128
                nc.gpsimd.dma_gather(
                    stages[st][:, co:co + ncols, :], d_hbuf[:],
                    hidx_sb[:, o8:o8 + ncols * 8],
                    num_idxs=ni, num_idxs_reg=ni,
                    elem_size=CH, single_packet=False,
                )
                o8 += ncols * 8

            if M > 0:
                for i, cb in enumerate(meta["cols_used"]):
                    half, lc = (0, cb) if cb < HALF else (1, cb - HALF)
                    nc.vector.copy_predicated(stages[half][:, lc, :],
                                              invm_sb[:, i, :], povs[i][:])

            nc.sync.dma_start(d_out[:, 0:HALF, :], stages[0][:])
            nc.scalar.dma_start(d_out[:, HALF:NCOL, :], stages[1][:])
    nc.compile()
    return nc


def _prog_key(meta):
    return ("prog", meta["M"], meta["NCHUNK"], meta["Tinv"],
            tuple(meta["n_lo"]), tuple(meta["n_hi"]), meta["cols_used"])


def _run(inputs, trace=False):
    meta, in_maps = _host_prep(**inputs)
    key = _prog_key(meta)
    if key not in _cache:
        _cache[key] = _build_program(meta)
    nc = _cache[key]
    res = run_bass_kernel_spmd(nc, in_maps, list(range(N_CORES)), trace=trace)
    out = np.concatenate(
        [res.results[c]["out"].transpose(1, 0, 2).reshape(NPAD, CH)[:DPC]
         for c in range(N_CORES)], axis=0
    ).astype(np.float32)
    return out, res


def kernel(**inputs):
    out, _ = _run(inputs)
    return out


# revision 42
# speedup vs baseline: 2.3637x; 1.0041x over previous
"""RGCN-with-history (DGL RelGraphConv + history splice) on 8 TRN2 NeuronCores.

Key structural fact: the history splice dominates — out[n] is an exact copy of
history_buffer[history_map[n]] wherever history_map[n] >= 0, and the RGCN
aggregation only survives for the (very few) nodes with history_map[n] < 0.

Strategy (memory-bound regime):
  - Shard destination nodes across 8 cores (6250 each); each core
    indirect-gathers its history rows straight into two output staging
    halves (two dma_gathers, pipelined with the two output DMAs).
  - The globally-rare "no history" nodes are computed on every core
    (replicated tiny fp32 compute keeps the SPMD program identical): their
    incoming edges are bucketed into 16-node chunks; per 128-edge tile we
    indirect-gather source features and accumulate Z^T[64, 128] += Xg^T @ S
    on the tensor engine, where S is a (relation, node-rank) one-hot built
    on the vector engine (is_equal against an iota row). Relation weights +
    self-loop + bias are applied with small matmuls.
  - Computed rows are routed to their data-dependent positions with one-hot
    selector matmuls (only for the few staging columns that contain such a
    node on any core) and overlaid onto the history staging via predicated
    copies. Everything stays on-chip; no DRAM round-trip.
"""
import sys

sys.path.insert(0, "/opt/trn_rl_repo")

import numpy as np

import concourse.bacc as bacc
import concourse.tile as tile
import concourse.mybir as mybir
from concourse.bass_utils import run_bass_kernel_spmd

N_NODES = 50000
N_EDGES = 800000
CH = 64
N_REL = 8
BUF = 20000
N_CORES = 8
DPC = N_NODES // N_CORES            # 6250 dst nodes per core
NPAD = 6400                         # padded dst rows per core (50 x 128)
NCOL = NPAD // 128                  # 50 staging columns
SPLIT = 32767                       # src < SPLIT -> lo table, else hi
T0_ROWS = SPLIT + 1                 # lo table rows; row SPLIT is zeros
T1_ROWS = N_NODES - SPLIT + 1       # hi table rows; row 0 is zeros
CHUNK = 16                          # invalid nodes per compute chunk
BATCH = 4096                        # max gather indices per dma_gather

_cache = {}


def _wrap16(a):
    """Flat index array -> [128, len/16] int16 wrapped layout (idx k at
    [k%16, k//16], replicated across the 8 gpsimd lanes)."""
    m = a.reshape(-1, 16).T.astype(np.int16)
    return np.tile(m, (8, 1)).copy()


def _host_prep(x, W, loop_w, bias, history_buffer, src, dst, etypes, history_map):
    src = np.asarray(src)
    dst = np.asarray(dst)
    etypes = np.asarray(etypes)
    x = np.asarray(x, dtype=np.float32)
    hm = np.asarray(history_map)
    hb = np.asarray(history_buffer, np.float32)

    # --- globally-rare invalid (no-history) nodes: replicated tiny compute ---
    inv_nodes = np.where(hm < 0)[0]              # sorted
    M = len(inv_nodes)
    NCHUNK = max(1, -(-M // CHUNK)) if M > 0 else 0
    MP = max(CHUNK, NCHUNK * CHUNK)              # scratch rows (>=16)

    n_lo = np.zeros(max(NCHUNK, 1), np.int64)
    n_hi = np.zeros(max(NCHUNK, 1), np.int64)
    idx_lo_slots = []
    idx_hi_slots = []
    srk_cols = None
    Tinv = 0
    chunk_tiles = []
    if M > 0:
        grank = np.full(N_NODES, -1, np.int64)
        grank[inv_nodes] = np.arange(M)
        emask = grank[dst] >= 0
        e_src = src[emask]
        e_et = etypes[emask]
        e_rank = grank[dst[emask]]
        e_chunk = e_rank // CHUNK
        e_half = (e_src >= SPLIT).astype(np.int64)
        e_col = e_et * CHUNK + (e_rank % CHUNK)  # one-hot col within chunk

        # host-side halo of the invalid edges' source features (the
        # sharding hint's "halo of remote source features"): per 128-edge
        # tile, a [128, CH] f32 block; pad edges are zero rows.
        srk_list = []
        xg_list = []
        for ch in range(NCHUNK):
            m = e_chunk == ch
            cnt = int(m.sum())
            n = -(-cnt // 128) if cnt else 0
            n_lo[ch] = n
            srkv = np.zeros(n * 128, np.float32)
            srkv[:cnt] = e_col[m]
            xgv = np.zeros((n * 128, CH), np.float32)
            xgv[:cnt] = x[e_src[m]]
            tl = []
            for t in range(n):
                srk_list.append(srkv[t * 128:(t + 1) * 128])
                xg_list.append(xgv[t * 128:(t + 1) * 128])
                tl.append((0, t))
            chunk_tiles.append(tl)
        Tinv = len(srk_list)
        srk_cols = (np.stack(srk_list, axis=1) if Tinv
                    else np.zeros((128, 0), np.float32))

    TinvP = max(1, Tinv)
    srk = np.zeros((128, TinvP), np.float32)
    xg_halo = np.zeros((128, TinvP, CH), np.float32)
    if Tinv:
        srk[:, :Tinv] = srk_cols
        for t, blk in enumerate(xg_list):
            xg_halo[:, t, :] = blk

    # union (over cores) of staging columns that hold an invalid node —
    # only these columns need the computed-row overlay
    if M:
        inv_local = inv_nodes % DPC
        cols_used = sorted(set((inv_local // 128).tolist()))
    else:
        cols_used = []

    meta = {
        "M": M, "NCHUNK": NCHUNK, "MP": MP, "Tinv": Tinv, "TinvP": TinvP,
        "n_lo": n_lo, "n_hi": n_hi, "chunk_tiles": chunk_tiles,
        "cols_used": tuple(cols_used),
    }

    # --- weights / constants (shared) ---
    Wsb = np.zeros((64, N_REL, CH), np.float32)
    for r in range(N_REL):
        Wsb[:, r, :] = np.asarray(W[r], np.float32)
    lwa = np.zeros((128, CH), np.float32)
    lwa[:CH] = np.asarray(loop_w, np.float32)
    lwa[CH] = np.asarray(bias, np.float32)
    iota = np.tile(np.arange(128, dtype=np.float32)[None, :], (128, 1)).copy()
    xti = np.zeros((128, MP), np.float32)
    if M:
        xti[:CH, :M] = x[inv_nodes].T
        xti[CH, :M] = 1.0

    # merge the small f32 constants into one array (fewer DMAs):
    # [srk | iota(128) | lwa(64) | xti(MP) | wsb(512, rows 0:64)]
    cmega = np.zeros((128, TinvP + 128 + CH + MP + N_REL * CH), np.float32)
    o = 0
    cmega[:, o:o + TinvP] = srk; o += TinvP
    cmega[:, o:o + 128] = iota; o += 128
    cmega[:, o:o + CH] = lwa; o += CH
    cmega[:, o:o + MP] = xti; o += MP
    cmega[:64, o:o + N_REL * CH] = Wsb.reshape(64, N_REL * CH)

    shared = {"cmega": cmega, "xg": xg_halo, "hbuf": hb}

    in_maps = []
    for c in range(N_CORES):
        hm_loc = np.zeros(NPAD, np.int64)
        hm_loc[:DPC] = hm[c * DPC:(c + 1) * DPC]
        hidx = np.clip(hm_loc, 0, BUF - 1)
        valid = hm_loc >= 0
        valid[DPC:] = True               # pad rows: treat as "history" side
        # selector + mask shipped only for the staging columns in cols_used
        NCU = max(len(cols_used), 1)
        sel = np.zeros((CHUNK, max(NCHUNK, 1) * NCU * 128), np.float32)
        invmask = np.zeros((128, NCU, CH), np.uint8)
        if M:
            gr = grank[c * DPC:(c + 1) * DPC]
            loc_inv = np.where(gr >= 0)[0]
            col_pos = {cb: i for i, cb in enumerate(cols_used)}
            for n in loc_inv:
                rr = int(gr[n])
                i = col_pos[n // 128]
                sel[rr % CHUNK,
                    ((rr // CHUNK) * NCU + i) * 128 + (n % 128)] = 1.0
            inv_full = (~valid).reshape(-1, 128).T
            for i, cb in enumerate(cols_used):
                invmask[:, i, :] = inv_full[:, cb][:, None]
        in_maps.append({
            **shared,
            "hidx": _wrap16(hidx), "sel": sel, "invmask": invmask,
        })
    return meta, in_maps


def _build_program(meta):
    M, NCHUNK, MP = meta["M"], meta["NCHUNK"], meta["MP"]
    TinvP = meta["TinvP"]
    CMW = TinvP + 128 + CH + MP + N_REL * CH
    HALF = NCOL // 2                     # staging split for pipelining

    nc = bacc.Bacc("TRN2", target_bir_lowering=False, debug=False,
                   num_devices=N_CORES,
                   # all gathers together emit ~14k SWDGE descriptors; the
                   # default 1024-descriptor ring forces a mid-kernel drain
                   dynamic_dma_scratch_size=1 << 17)
    dt = mybir.dt
    d_cm = nc.dram_tensor("cmega", [128, CMW], dt.float32, kind="ExternalInput")
    d_xg = nc.dram_tensor("xg", [128, TinvP, CH], dt.float32,
                          kind="ExternalInput")
    d_hbuf = nc.dram_tensor("hbuf", [BUF, CH], dt.float32, kind="ExternalInput")
    d_hidx = nc.dram_tensor("hidx", [128, NPAD // 16], dt.int16, kind="ExternalInput")
    NCU = max(len(meta["cols_used"]), 1)
    d_sel = nc.dram_tensor("sel", [CHUNK, max(NCHUNK, 1) * NCU * 128],
                           dt.float32, kind="ExternalInput")
    d_invm = nc.dram_tensor("invmask", [128, NCU, CH], dt.uint8,
                            kind="ExternalInput")
    d_out = nc.dram_tensor("out", [128, NCOL, CH], dt.float32, kind="ExternalOutput")

    with tile.TileContext(nc) as tc:
        with (
            tc.tile_pool(name="const", bufs=1) as cpool,
            tc.tile_pool(name="g", bufs=2) as gpool,
            tc.tile_pool(name="s", bufs=2) as spool,
            tc.tile_pool(name="pz", bufs=2, space="PSUM") as pzpool,
            tc.tile_pool(name="po", bufs=2, space="PSUM") as popool,
            tc.tile_pool(name="pov", bufs=4, space="PSUM") as povpool,
        ):
            hidx_sb = cpool.tile([128, NPAD // 16], dt.int16)
            # two staging halves -> history gather and output DMA pipeline
            stages = [cpool.tile([128, HALF, CH], dt.float32, name="stageA"),
                      cpool.tile([128, NCOL - HALF, CH], dt.float32,
                                 name="stageB")]

            if M > 0:
                xg_sb = cpool.tile([128, TinvP, CH], dt.float32)
                cm_sb = cpool.tile([128, CMW], dt.float32)
                sel_sb = cpool.tile([CHUNK, max(NCHUNK, 1) * NCU * 128],
                                    dt.float32)
                invm_sb = cpool.tile([128, NCU, CH], dt.uint8)
                # const DMA issue order controls when history desc-gen can
                # start (hidx first) vs. when the invalid-node compute chain
                # has its operands (tuned against the modeled timeline)
                for eng, pairs in (
                        (nc.sync, ((hidx_sb, d_hidx), (xg_sb, d_xg),
                                   (sel_sb, d_sel))),
                        (nc.scalar, ((cm_sb, d_cm), (invm_sb, d_invm)))):
                    for t_sb, t_d in pairs:
                        eng.dma_start(t_sb[:], t_d[:])
                o = 0
                srk_sb = cm_sb[:, 0:TinvP]; o = TinvP
                iota_sb = cm_sb[:, o:o + 128]; o += 128
                lwa_sb = cm_sb[:, o:o + CH]; o += CH
                xti_sb = cm_sb[:, o:o + MP]; o += MP
                wsb_o = o

                gt = 0
                cps = []
                for ch in range(NCHUNK):
                    tl = meta["chunk_tiles"][ch]
                    ntot = len(tl)
                    if ntot:
                        pz = pzpool.tile([64, 128], dt.float32, tag="pz",
                                         name=f"pz_{ch}")
                        for i, (h, t) in enumerate(tl):
                            S = spool.tile([128, 128], dt.float32, tag="S",
                                           name=f"S_{ch}_{i}")
                            nc.vector.tensor_scalar(
                                S[:], iota_sb, srk_sb[:, gt:gt + 1], None,
                                mybir.AluOpType.is_equal,
                            )
                            nc.tensor.matmul(pz[:], xg_sb[:, gt, :], S[:],
                                             start=(i == 0),
                                             stop=(i == ntot - 1))
                            gt += 1
                        zt = spool.tile([64, 128], dt.float32, tag="zt",
                                        name=f"zt_{ch}")
                        nc.scalar.activation(zt[:], pz[:],
                                             mybir.ActivationFunctionType.Copy)
                    po = popool.tile([CHUNK, CH], dt.float32, tag="po",
                                     name=f"po_{ch}")
                    nc.tensor.matmul(po[:], xti_sb[:, ch * CHUNK:(ch + 1) * CHUNK],
                                     lwa_sb, start=True, stop=(ntot == 0))
                    if ntot:
                        for r in range(N_REL):
                            nc.tensor.matmul(
                                po[:], zt[:, r * CHUNK:(r + 1) * CHUNK],
                                cm_sb[0:64, wsb_o + r * CH:wsb_o + (r + 1) * CH],
                                start=False, stop=(r == N_REL - 1),
                            )
                    cp = cpool.tile([CHUNK, CH], dt.float32,
                                    name=f"cp_{ch}")
                    nc.vector.tensor_copy(cp[:], po[:])
                    cps.append(cp)

                # route computed rows to their positions; only columns that
                # hold an invalid node on some core need the overlay
                povs = []
                for i, cb in enumerate(meta["cols_used"]):
                    pov = povpool.tile([128, CH], dt.float32, tag="pov",
                                       name=f"pov_{cb}")
                    for ch in range(NCHUNK):
                        nc.tensor.matmul(
                            pov[:],
                            sel_sb[:, (ch * NCU + i) * 128:
                                   (ch * NCU + i) * 128 + 128],
                            cps[ch][:], start=(ch == 0),
                            stop=(ch == NCHUNK - 1),
                        )
                    povs.append(pov)

            if M == 0:
                nc.sync.dma_start(hidx_sb[:], d_hidx[:])
            # history gathers: a small head segment first so its (short)
            # desc-gen completes early and transfers start sooner; later
            # segments' desc-gen pipelines behind running transfers
            segs = ((0, 0, 10), (0, 10, HALF - 10), (1, 0, NCOL - HALF))
            o8 = 0
            for st, co, ncols in segs:
                ni = ncols * 128
                nc.gpsimd.dma_gather(
                    stages[st][:, co:co + ncols, :], d_hbuf[:],
                    hidx_sb[:, o8:o8 + ncols * 8],
                    num_idxs=ni, num_idxs_reg=ni,
                    elem_size=CH, single_packet=False,
                )
                o8 += ncols * 8

            if M > 0:
                for i, cb in enumerate(meta["cols_used"]):
                    half, lc = (0, cb) if cb < HALF else (1, cb - HALF)
                    nc.vector.copy_predicated(stages[half][:, lc, :],
                                              invm_sb[:, i, :], povs[i][:])

            nc.sync.dma_start(d_out[:, 0:HALF, :], stages[0][:])
            nc.scalar.dma_start(d_out[:, HALF:NCOL, :], stages[1][:])
    nc.compile()
    return nc


def _prog_key(meta):
    return ("prog", meta["M"], meta["NCHUNK"], meta["Tinv"],
            tuple(meta["n_lo"]), tuple(meta["n_hi"]), meta["cols_used"])


def _run(inputs, trace=False):
    meta, in_maps = _host_prep(**inputs)
    key = _prog_key(meta)
    if key not in _cache:
        _cache[key] = _build_program(meta)
    nc = _cache[key]
    res = run_bass_kernel_spmd(nc, in_maps, list(range(N_CORES)), trace=trace)
    out = np.concatenate(
        [res.results[c]["out"].transpose(1, 0, 2).reshape(NPAD, CH)[:DPC]
         for c in range(N_CORES)], axis=0
    ).astype(np.float32)
    return out, res


def kernel(**inputs):
    out, _ = _run(inputs)
    return out
